# revision 1
# baseline (speedup 1.0000x reference)
"""Trainium2 Bass kernel for nn_CrossAttention (B=4, Lq=Lk=2048, D=1024, H=16, d=64).

Sharding: 8 cores = 4 batches x 2 head-groups (8 heads each).
Each core computes a partial out^T = Wout_g^T @ y_g^T for its (batch, head-group);
host sums the two head-group partials per batch and transposes.

Device layout is feature-major ("T" = [feature, seq]) throughout:
  qT/kT: [512, L] (8 heads x 64 dims on partitions, seq on free axis)
  S^T:   [k, q] tiles -> softmax sum via an appended ones-column in v (M=65)
  exp:   ACT, with the k-side RMSNorm rstd (and the 1/sqrt(d) scale) folded
         into the per-partition activation scale operand.
"""
import os
import numpy as np
from contextlib import ExitStack

import concourse.bass as bass
import concourse.tile as tile
from concourse import bacc, mybir
from concourse.bass_utils import run_bass_kernel_spmd

F32 = mybir.dt.float32
F32R = mybir.dt.float32r
BF16 = mybir.dt.bfloat16
EXP = mybir.ActivationFunctionType.Exp
SQUARE = mybir.ActivationFunctionType.Square
SQRT = mybir.ActivationFunctionType.Sqrt

D = 1024          # model dim
L = 2048          # seq len (q and k)
HC = 8            # heads per core
DH = 64           # head dim
F = HC * DH       # 512 local features
N_CORES = 8
EPS = float(np.finfo(np.float32).eps)

LAST_RESULTS = None  # BassKernelResults of the most recent run (for test harness)
_NC = None


# --------------------------------------------------------------------------- #
# Device program
# --------------------------------------------------------------------------- #

def _proj_norm_rope(tc, ctx, dst, x_dram, w_dram, c_dram, s_dram, bdiag, bmap,
                    side, rk_dram=None, wv_dram=None, vaug=None):
    """Project x (via w) into feature-major dst tiles [128, L] x4, then apply
    RMSNorm + RoPE in place.

    side == "q": multiply rstd into dst (via broadcast matmul).
    side == "k": write 0.125*rstd chunks to rk_dram instead (consumed by exp).
    """
    nc = tc.nc
    pool = ctx.enter_context(tc.tile_pool(name=f"{side}_sb", bufs=1))
    pps = ctx.enter_context(tc.tile_pool(name=f"{side}_ps", bufs=1, space="PSUM"))

    # rope tables [128, L]
    c_sb = pool.tile([128, L], F32, tag="ctab")
    nc.sync.dma_start(c_sb[:], c_dram[:])
    s_sb = pool.tile([128, L], F32, tag="stab")
    nc.sync.dma_start(s_sb[:], s_dram[:])
    # weights [128, F] x8
    w_sb = []
    for dc in range(8):
        w = pool.tile([128, F], F32R, tag=f"w{dc}")
        nc.sync.dma_start(w[:], w_dram[dc * 128:(dc + 1) * 128, :])
        w_sb.append(w)

    eps_t = pool.tile([2, 1], F32, tag="eps", name=f"eps_{side}")
    nc.gpsimd.memset(eps_t[:], EPS if side == "q" else 64.0 * EPS)
    wv_sb = []
    if wv_dram is not None:
        for dc in range(8):
            w = pool.tile([128, F], F32R, tag=f"wv{dc}", name=f"wv_sb{dc}")
            nc.sync.dma_start(w[:], wv_dram[dc * 128:(dc + 1) * 128, :])
            wv_sb.append(w)

    for lh in range(2):           # seq halves (SBUF residency)
        x_sb = []
        for dc in range(8):
            x = pool.tile([128, 1024], F32R, tag=f"x{dc}", bufs=1)
            nc.sync.dma_start(x[:], x_dram[dc * 128:(dc + 1) * 128,
                                           lh * 1024:(lh + 1) * 1024])
            x_sb.append(x)
        for fb in range(4):       # feature block: heads (2fb, 2fb+1)
            for qn in range(2):   # 512-wide seq chunk within the half
                col0 = lh * 1024 + qn * 512
                ps = pps.tile([128, 512], F32, tag="proj", bufs=3)
                for dc in range(8):
                    nc.tensor.matmul(
                        ps[:],
                        w_sb[dc][:, fb * 128:(fb + 1) * 128].bitcast(F32R),
                        x_sb[dc][:, qn * 512:(qn + 1) * 512].bitcast(F32R),
                        start=(dc == 0), stop=(dc == 7))
                chunk = dst[fb][:, col0:col0 + 512]
                # evict raw projection
                nc.vector.tensor_copy(chunk, ps[:])
                # sum of squares over each head's 64 partition rows
                sq = pool.tile([128, 512], F32R, tag="sq", bufs=2)
                nc.scalar.activation(sq[:], ps[:], SQUARE)
                vps = pps.tile([2, 512], F32, tag="var", bufs=2)
                nc.tensor.matmul(vps[:], bdiag.bitcast(F32R), sq.bitcast(F32R),
                                 start=True, stop=True)
                std = pool.tile([2, 512], F32, tag="std", bufs=2)
                rstd = pool.tile([2, 512], F32, tag="rstd", bufs=2)
                if side == "q":
                    # std = sqrt(raw/64 + eps); rstd = 1/std
                    nc.scalar.activation(std[:], vps[:], SQRT,
                                         bias=eps_t[:], scale=1.0 / 64.0)
                    nc.vector.reciprocal_approx_fast(out=rstd[:], in_=std[:])
                    rstd_r = pool.tile([2, 512], F32R, tag="rstdr", bufs=2)
                    nc.vector.tensor_copy(rstd_r[:], rstd[:])
                else:
                    # fold the 1/8 attention scale: rk = 1/(8*std) = 1/sqrt(64*(raw/64+eps))
                    nc.scalar.activation(std[:], vps[:], SQRT,
                                         bias=eps_t[:], scale=1.0)
                    nc.vector.reciprocal_approx_fast(out=rstd[:], in_=std[:])
                    nc.sync.dma_start(
                        rk_dram[2 * fb:2 * fb + 2, col0:col0 + 512], rstd[:])
                # rotate-half copies (gpsimd; reads raw chunk before in-place ops)
                tmp = pool.tile([128, 512], F32, tag="tmp", bufs=2)
                nc.gpsimd.tensor_copy(tmp[0:32, :], chunk[32:64, :])
                nc.gpsimd.tensor_copy(tmp[32:64, :], chunk[0:32, :])
                nc.gpsimd.tensor_copy(tmp[64:96, :], chunk[96:128, :])
                nc.gpsimd.tensor_copy(tmp[96:128, :], chunk[64:96, :])
                # rope combine: chunk = chunk*C + tmp*S
                nc.vector.tensor_mul(chunk, chunk, c_sb[:, col0:col0 + 512])
                nc.vector.tensor_mul(tmp[:], tmp[:], s_sb[:, col0:col0 + 512])
                nc.vector.tensor_add(chunk, chunk, tmp[:])
                if side == "q":
                    bps = pps.tile([128, 512], F32, tag="bc", bufs=2)
                    nc.tensor.matmul(bps[:], bmap.bitcast(F32R),
                                     rstd_r.bitcast(F32R), start=True, stop=True)
                    nc.vector.tensor_mul(chunk, chunk, bps[:])
        if wv_dram is not None:
            for lc in range(8):
                kc = lh * 8 + lc
                ps = pps.tile([128, 512], F32, tag="proj", bufs=3)
                for dc in range(8):
                    nc.tensor.matmul(
                        ps[:],
                        x_sb[dc][:, lc * 128:(lc + 1) * 128].bitcast(F32R),
                        wv_sb[dc].bitcast(F32R),
                        start=(dc == 0), stop=(dc == 7))
                va = vaug[kc]
                nc.gpsimd.memset(va[:], 1.0)
                va3 = va.rearrange("p (h c) -> p h c", c=65)
                ps3 = ps.rearrange("p (h c) -> p h c", c=64)
                nc.vector.tensor_copy(va3[:, :, 0:64], ps3[:])


def _build_program():
    nc = bacc.Bacc("TRN2", target_bir_lowering=False, debug=False,
                   num_devices=N_CORES)
    dt = nc.dram_tensor
    xqT = dt("xqT", (D, L), F32R, kind="ExternalInput").ap()
    xkvT = dt("xkvT", (D, L), F32R, kind="ExternalInput").ap()
    wq = dt("wq", (D, F), F32R, kind="ExternalInput").ap()
    wk = dt("wk", (D, F), F32R, kind="ExternalInput").ap()
    wv = dt("wv", (D, F), F32R, kind="ExternalInput").ap()
    wout = dt("wout", (F, D), F32R, kind="ExternalInput").ap()
    cq = dt("cq", (128, L), F32, kind="ExternalInput").ap()
    sq_t = dt("sq", (128, L), F32, kind="ExternalInput").ap()
    ck = dt("ck", (128, L), F32, kind="ExternalInput").ap()
    sk_t = dt("sk", (128, L), F32, kind="ExternalInput").ap()
    bdiag_d = dt("bdiag", (128, 2), F32R, kind="ExternalInput").ap()
    bmap_d = dt("bmap", (2, 128), F32R, kind="ExternalInput").ap()
    selA_d = dt("selA", (128, 128), F32R, kind="ExternalInput").ap()
    selB_d = dt("selB", (128, 128), F32R, kind="ExternalInput").ap()
    outT = dt("outT", (D, L), F32, kind="ExternalOutput").ap()

    with tile.TileContext(nc) as tc:
        with ExitStack() as ctx:
            big = ctx.enter_context(tc.tile_pool(name="big", bufs=1))
            dram = ctx.enter_context(tc.tile_pool(name="dram", bufs=1, space="DRAM"))

            kT = [big.tile([128, L], F32R, tag=f"kT{i}", name=f"kT{i}") for i in range(4)]
            qT = [big.tile([128, L], F32R, tag=f"qT{i}", name=f"qT{i}") for i in range(4)]
            vaug = [big.tile([128, HC * 65], BF16, tag=f"v{i}", name=f"vaug{i}") for i in range(16)]
            rk_dram = dram.tile([HC, L], F32, tag="rk")

            bdiag = big.tile([128, 2], F32R, tag="bdiag")
            nc.sync.dma_start(bdiag[:], bdiag_d[:])
            bmap = big.tile([2, 128], F32R, tag="bmap")
            nc.sync.dma_start(bmap[:], bmap_d[:])

            # ---- Phase A: kv projections (k feature-major + v into vaug) ----
            with ExitStack() as actx:
                _proj_norm_rope(tc, actx, kT, xkvT, wk, ck, sk_t, bdiag, bmap,
                                side="k", rk_dram=rk_dram, wv_dram=wv,
                                vaug=vaug)

            # ---- Phase B: q projection ----
            with ExitStack() as bctx:
                _proj_norm_rope(tc, bctx, qT, xqT, wq, cq, sq_t, bdiag, bmap,
                                side="q")

            # ---- Phases C+D persistents ----
            p2 = ctx.enter_context(tc.tile_pool(name="p2", bufs=1))
            ytr = [p2.tile([128, L], F32R, tag=f"ytr{i}", name=f"ytr{i}")
                   for i in range(4)]
            sums_g = [p2.tile([128, L], F32, tag=f"sums{g}", name=f"sums{g}")
                      for g in range(2)]
            nc.gpsimd.memset(sums_g[0][:], 1.0)
            nc.gpsimd.memset(sums_g[1][:], 1.0)

            # ---- Phase C: attention ----
            with ExitStack() as cctx:
                cpool = cctx.enter_context(tc.tile_pool(name="att_sb", bufs=1))
                cps = cctx.enter_context(
                    tc.tile_pool(name="att_ps", bufs=1, space="PSUM"))
                rk_sb = cpool.tile([128, HC, 16], F32, tag="rk")
                nc.sync.dma_start(
                    rk_sb[:], rk_dram.rearrange("h (kc p) -> p h kc", p=128))
                for h in range(HC):
                    fb, off = h // 2, (h % 2) * 64
                    yps = [cps.tile([65, 512], F32, tag=f"y{qn}", bufs=1,
                                     name=f"yps{h}_{qn}")
                           for qn in range(4)]
                    for kc in range(16):
                        va3 = vaug[kc].rearrange("p (h c) -> p h c", c=65)
                        rk_ap = rk_sb[:, h, kc:kc + 1]
                        for half in range(2):
                            sps = cps.tile([128, 1024], F32, tag="s", bufs=2)
                            for j in range(2):
                                qn = half * 2 + j
                                nc.tensor.matmul(
                                    sps[:, j * 512:(j + 1) * 512],
                                    kT[fb][off:off + 64,
                                           kc * 128:(kc + 1) * 128].bitcast(F32R),
                                    qT[fb][off:off + 64,
                                           qn * 512:(qn + 1) * 512].bitcast(F32R),
                                    start=True, stop=True)
                            pt = cpool.tile([128, 1024], BF16, tag="p", bufs=3)
                            nc.scalar.activation(pt[:], sps[:], EXP, scale=rk_ap)
                            for j in range(2):
                                qn = half * 2 + j
                                nc.tensor.matmul(
                                    yps[qn][:], va3[:, h, :],
                                    pt[:, j * 512:(j + 1) * 512],
                                    start=(kc == 0), stop=(kc == 15))
                    for qn in range(4):
                        nc.vector.tensor_copy(
                            ytr[fb][off:off + 64, qn * 512:(qn + 1) * 512],
                            yps[qn][0:64, :])
                        slot = 32 * (h % 4)
                        nc.vector.tensor_copy(
                            sums_g[h // 4][slot:slot + 1,
                                           qn * 512:(qn + 1) * 512],
                            yps[qn][64:65, :])

            # ---- Phase D: normalize + output projection ----
            with ExitStack() as dctx:
                dpool = dctx.enter_context(tc.tile_pool(name="out_sb", bufs=1))
                dps = dctx.enter_context(
                    tc.tile_pool(name="out_ps", bufs=1, space="PSUM"))
                wo_sb = []
                for fc in range(4):
                    w = dpool.tile([128, D], F32R, tag=f"wo{fc}")
                    nc.sync.dma_start(w[:], wout[fc * 128:(fc + 1) * 128, :])
                    wo_sb.append(w)
                sel_sb = []
                for i, sd in enumerate((selA_d, selB_d)):
                    s = dpool.tile([128, 128], F32R, tag=f"sel{i}",
                                   name=f"sel{i}")
                    nc.sync.dma_start(s[:], sd[:])
                    sel_sb.append(s)
                rs_g = []
                for g in range(2):
                    rs32 = dpool.tile([128, L], F32, tag="rs32", bufs=2,
                                      name=f"rs32_{g}")
                    nc.vector.reciprocal_approx_fast(
                        out=rs32[:], in_=sums_g[g][:])
                    rs = dpool.tile([128, L], F32R, tag="rs", bufs=2,
                                    name=f"rs{g}")
                    nc.vector.tensor_copy(rs[:], rs32[:])
                    rs_g.append(rs)
                for fb in range(4):
                    sel = sel_sb[fb % 2]
                    bps = dps.tile([128, L], F32, tag="bc2", bufs=1)
                    for qn in range(4):
                        nc.tensor.matmul(
                            bps[:, qn * 512:(qn + 1) * 512],
                            sel.bitcast(F32R),
                            rs_g[fb // 2][:, qn * 512:(qn + 1) * 512]
                            .bitcast(F32R),
                            start=True, stop=True)
                    nc.vector.tensor_mul(ytr[fb][:], ytr[fb][:], bps[:])
                for nb in range(8):
                    for qn in range(4):
                        ps = dps.tile([128, 512], F32, tag="oproj", bufs=3)
                        for fc in range(4):
                            nc.tensor.matmul(
                                ps[:],
                                wo_sb[fc][:, nb * 128:(nb + 1) * 128].bitcast(F32R),
                                ytr[fc][:, qn * 512:(qn + 1) * 512].bitcast(F32R),
                                start=(fc == 0), stop=(fc == 3))
                        ot = dpool.tile([128, 512], F32, tag="ot", bufs=3)
                        nc.vector.tensor_copy(ot[:], ps[:])
                        nc.sync.dma_start(
                            outT[nb * 128:(nb + 1) * 128,
                                 qn * 512:(qn + 1) * 512], ot[:])
    nc.compile()
    return nc


def get_nc():
    global _NC
    if _NC is None:
        _NC = _build_program()
    return _NC


# --------------------------------------------------------------------------- #
# Host side
# --------------------------------------------------------------------------- #

def _rope_tables(pos, g):
    """Feature-major folded RoPE(+gain) tables, replicated for a 2-head tile."""
    pos = np.asarray(pos).astype(np.float32)
    g = np.asarray(g, dtype=np.float32)
    inv = (1.0 / (10000.0 ** (np.arange(0, DH, 2, dtype=np.float32)
                              / np.float32(DH)))).astype(np.float32)
    ang = pos[:, None] * inv[None, :]                      # (L, 32)
    cos, sin = np.cos(ang, dtype=np.float32), np.sin(ang, dtype=np.float32)
    j = np.arange(DH)
    C = (g[j][:, None] * cos[:, j % 32].T).astype(np.float32)       # (64, L)
    sign = np.where(j < 32, -1.0, 1.0).astype(np.float32)
    S = (sign[:, None] * g[(j + 32) % 64][:, None]
         * sin[:, j % 32].T).astype(np.float32)
    return (np.ascontiguousarray(np.tile(C, (2, 1))),
            np.ascontiguousarray(np.tile(S, (2, 1))))     # (128, L) each


def make_in_maps(queries, kv, Wq, Wkv, Wout, g_q, g_k, pos_q, pos_k):
    queries = np.asarray(queries, dtype=np.float32)
    kv = np.asarray(kv, dtype=np.float32)
    Wq = np.asarray(Wq, dtype=np.float32)
    Wkv = np.asarray(Wkv, dtype=np.float32)
    Wout = np.asarray(Wout, dtype=np.float32)

    cq, sq = _rope_tables(pos_q, g_q)
    ck, sk = _rope_tables(pos_k, g_k)
    bdiag = np.zeros((128, 2), np.float32)
    bdiag[0:64, 0] = 1.0
    bdiag[64:128, 1] = 1.0
    bmap = np.zeros((2, 128), np.float32)
    bmap[0, 0:64] = 1.0
    bmap[1, 64:128] = 1.0
    selA = np.zeros((128, 128), np.float32)
    selA[0, 0:64] = 1.0
    selA[32, 64:128] = 1.0
    selB = np.zeros((128, 128), np.float32)
    selB[64, 0:64] = 1.0
    selB[96, 64:128] = 1.0

    Wkv3 = Wkv.reshape(D, 16, 2 * DH)
    in_maps = []
    for c in range(N_CORES):
        b, grp = c // 2, c % 2
        hs = slice(grp * HC, (grp + 1) * HC)
        in_maps.append({
            "xqT": np.ascontiguousarray(queries[b].T),
            "xkvT": np.ascontiguousarray(kv[b].T),
            "wq": np.ascontiguousarray(Wq[:, grp * F:(grp + 1) * F]),
            "wk": np.ascontiguousarray(Wkv3[:, hs, :DH].reshape(D, F)),
            "wv": np.ascontiguousarray(Wkv3[:, hs, DH:].reshape(D, F)),
            "wout": np.ascontiguousarray(Wout[grp * F:(grp + 1) * F, :]),
            "cq": cq, "sq": sq, "ck": ck, "sk": sk,
            "bdiag": bdiag, "bmap": bmap, "selA": selA, "selB": selB,
        })
    return in_maps


def kernel(queries, kv, Wq, Wkv, Wout, g_q, g_k, pos_q, pos_k):
    global LAST_RESULTS
    nc = get_nc()
    in_maps = make_in_maps(queries, kv, Wq, Wkv, Wout, g_q, g_k, pos_q, pos_k)
    trace = bool(int(os.environ.get("KERNEL_TRACE", "0")))
    kw = {}
    if trace:
        kw["tmpdir"] = os.environ.get("KERNEL_TRACE_DIR") or None
    res = run_bass_kernel_spmd(nc, in_maps, core_ids=list(range(N_CORES)),
                               trace=trace, **kw)
    LAST_RESULTS = res
    out = np.empty((4, L, D), np.float32)
    for b in range(4):
        out[b] = (res.results[2 * b]["outT"]
                  + res.results[2 * b + 1]["outT"]).T
    return out



# revision 2
# speedup vs baseline: 1.6023x; 1.6023x over previous
"""Trainium2 Bass kernel for nn_CrossAttention (B=4, Lq=Lk=2048, D=1024, H=16, d=64).

Sharding: 8 cores = 4 batches x 2 head-groups (8 heads each).
Each core computes a partial out^T = Wout_g^T @ y_g^T for its (batch, head-group);
host sums the two head-group partials per batch and transposes.

v2: all matmuls run in bf16 (f32r matmuls cost ~2x bf16 on HW), and the RoPE
rotate-half moved from GpSimd partition copies onto the PE as a permutation
matmul. Projection phases are software-pipelined (proj c | perm/var c-1 |
bcast c-2) so the PE never waits on the vector/scalar RMSNorm chain.

Device layout is feature-major ("T" = [feature, seq]) throughout:
  qT/kT: [512, L] bf16 (8 heads x 64 dims on partitions, seq on free axis)
  S^T:   [k, q] tiles -> softmax sum via an appended ones-column in v (M=65)
  exp:   ACT, with the k-side RMSNorm rstd (and the 1/sqrt(d) scale) folded
         into the per-partition activation scale operand.
"""
import os
import numpy as np
from contextlib import ExitStack

import concourse.bass as bass
import concourse.tile as tile
from concourse import bacc, mybir
from concourse.bass_utils import run_bass_kernel_spmd

F32 = mybir.dt.float32
BF16 = mybir.dt.bfloat16
NP_BF16 = mybir.dt.np(BF16)
EXP = mybir.ActivationFunctionType.Exp
SQUARE = mybir.ActivationFunctionType.Square
SQRT = mybir.ActivationFunctionType.Sqrt
COPYF = mybir.ActivationFunctionType.Copy

D = 1024          # model dim
L = 2048          # seq len (q and k)
HC = 8            # heads per core
DH = 64           # head dim
F = HC * DH       # 512 local features
N_CORES = 8
EPS = float(np.finfo(np.float32).eps)

LAST_RESULTS = None  # BassKernelResults of the most recent run (for test harness)
_NC = None


# --------------------------------------------------------------------------- #
# Device program
# --------------------------------------------------------------------------- #

def _proj_side(tc, ctx, dst, x_dram, w_dram, c_dram, s_dram, bdiag, bmap, perm,
               side, rk_dram=None, wv_dram=None, vaug=None):
    """Project x (via w) into feature-major bf16 dst tiles [128, L] x4, with
    RMSNorm + RoPE applied. Software-pipelined over 16 chunks [128, 512].

    side == "q": multiply rstd into dst (via broadcast matmul).
    side == "k": write 0.125*rstd chunks to rk_dram instead (consumed by exp),
                 and also project v (wv_dram) into vaug tiles.
    """
    nc = tc.nc
    pool = ctx.enter_context(tc.tile_pool(name=f"{side}_sb", bufs=1))
    pps = ctx.enter_context(tc.tile_pool(name=f"{side}_ps", bufs=1, space="PSUM"))

    # rope tables [128, L] bf16
    c_sb = pool.tile([128, L], BF16, tag="ctab")
    nc.sync.dma_start(c_sb[:], c_dram[:])
    s_sb = pool.tile([128, L], BF16, tag="stab")
    nc.sync.dma_start(s_sb[:], s_dram[:])
    # weights [128, F] x8 bf16
    w_sb = []
    for dc in range(8):
        w = pool.tile([128, F], BF16, tag=f"w{dc}")
        nc.sync.dma_start(w[:], w_dram[dc * 128:(dc + 1) * 128, :])
        w_sb.append(w)
    eps_t = pool.tile([2, 1], F32, tag="eps", name=f"eps_{side}")
    nc.gpsimd.memset(eps_t[:], EPS if side == "q" else 64.0 * EPS)
    wv_sb = []
    if wv_dram is not None:
        for dc in range(8):
            w = pool.tile([128, F], BF16, tag=f"wv{dc}", name=f"wv_sb{dc}")
            nc.sync.dma_start(w[:], wv_dram[dc * 128:(dc + 1) * 128, :])
            wv_sb.append(w)
    # full input, feature-major bf16 [128, 2048] x8
    x_sb = []
    for dc in range(8):
        x = pool.tile([128, L], BF16, tag=f"x{dc}")
        nc.sync.dma_start(x[:], x_dram[dc * 128:(dc + 1) * 128, :])
        x_sb.append(x)

    chunks = [(fb, qc) for fb in range(4) for qc in range(4)]
    st = [dict() for _ in chunks]   # per-chunk pipeline state

    def stage1(c):
        fb, qc = chunks[c]
        col0 = qc * 512
        ps = pps.tile([128, 512], F32, tag="proj", bufs=2)
        for dc in range(8):
            nc.tensor.matmul(ps[:],
                             w_sb[dc][:, fb * 128:(fb + 1) * 128],
                             x_sb[dc][:, col0:col0 + 512],
                             start=(dc == 0), stop=(dc == 7))
        raw = pool.tile([128, 512], BF16, tag="raw", bufs=3)
        nc.vector.tensor_copy(raw[:], ps[:])          # cast for perm matmul
        sq = pool.tile([128, 512], BF16, tag="sq", bufs=2)
        nc.scalar.activation(sq[:], ps[:], SQUARE)
        t1 = pool.tile([128, 512], BF16, tag="t1", bufs=3)
        nc.gpsimd.tensor_mul(t1[:], raw[:], c_sb[:, col0:col0 + 512])
        st[c].update(ps=ps, raw=raw, sq=sq, t1=t1, col0=col0, fb=fb)

    def stage1v(kc):
        # v projection chunk kc -> vaug[kc] (seq-major), k side only
        ps = pps.tile([128, 512], F32, tag="vproj", bufs=2)
        for dc in range(8):
            nc.tensor.matmul(ps[:],
                             x_sb[dc][:, kc * 128:(kc + 1) * 128],
                             wv_sb[dc][:],
                             start=(dc == 0), stop=(dc == 7))
        va = vaug[kc]
        nc.gpsimd.memset(va[:], 1.0)
        va3 = va.rearrange("p (h c) -> p h c", c=65)
        ps3 = ps.rearrange("p (h c) -> p h c", c=64)
        nc.vector.tensor_copy(va3[:, :, 0:64], ps3[:])

    def stage2(c):
        s = st[c]
        fb, col0 = s["fb"], s["col0"]
        rot = pps.tile([128, 512], F32, tag="rot", bufs=2)
        nc.tensor.matmul(rot[:], perm[:], s["raw"][:], start=True, stop=True)
        vps = pps.tile([2, 512], F32, tag="var", bufs=2)
        nc.tensor.matmul(vps[:], bdiag[:], s["sq"][:], start=True, stop=True)
        std = pool.tile([2, 512], F32, tag="std", bufs=2)
        if side == "q":
            # std = sqrt(raw/64 + eps); rstd = 1/std
            nc.scalar.activation(std[:], vps[:], SQRT,
                                 bias=eps_t[:], scale=1.0 / 64.0)
        else:
            # fold the 1/8 attention scale: rk = 1/(8*std) = 1/sqrt(64*(raw/64+eps))
            nc.scalar.activation(std[:], vps[:], SQRT,
                                 bias=eps_t[:], scale=1.0)
        t2 = pool.tile([128, 512], BF16, tag="t2", bufs=2)
        nc.vector.tensor_mul(t2[:], rot[:], s_sb[:, col0:col0 + 512])
        if side == "q":
            rstd = pool.tile([2, 512], F32, tag="rstd", bufs=3)
            nc.vector.reciprocal_approx_fast(out=rstd[:], in_=std[:])
            rstd_b = pool.tile([2, 512], BF16, tag="rstdb", bufs=3)
            nc.scalar.activation(rstd_b[:], rstd[:], COPYF)
            pre = pool.tile([128, 512], BF16, tag="pre", bufs=3)
            nc.vector.tensor_add(pre[:], s["t1"][:], t2[:])
            s.update(rstd_b=rstd_b, pre=pre)
        else:
            rstd = pool.tile([2, 512], F32, tag="rstd", bufs=2)
            nc.vector.reciprocal_approx_fast(out=rstd[:], in_=std[:])
            nc.sync.dma_start(
                rk_dram[2 * fb:2 * fb + 2, col0:col0 + 512], rstd[:])
            nc.vector.tensor_add(dst[fb][:, col0:col0 + 512], s["t1"][:], t2[:])

    def stage3(c):
        # q only: broadcast rstd over the 2x64 head rows and multiply in
        s = st[c]
        fb, col0 = s["fb"], s["col0"]
        bps = pps.tile([128, 512], F32, tag="bc", bufs=2)
        nc.tensor.matmul(bps[:], bmap[:], s["rstd_b"][:], start=True, stop=True)
        nc.vector.tensor_mul(dst[fb][:, col0:col0 + 512], s["pre"][:], bps[:])
        st[c] = {}

    n = len(chunks)
    if side == "k":
        for i in range(n + 1):
            if i < n:
                stage1(i)
                stage1v(i)
            if i >= 1:
                stage2(i - 1)
    else:
        for i in range(n + 2):
            if i < n:
                stage1(i)
            if 1 <= i <= n:
                stage2(i - 1)
            if i >= 2:
                stage3(i - 2)


def _build_program():
    nc = bacc.Bacc("TRN2", target_bir_lowering=False, debug=False,
                   num_devices=N_CORES)
    dt = nc.dram_tensor
    xqT = dt("xqT", (D, L), BF16, kind="ExternalInput").ap()
    xkvT = dt("xkvT", (D, L), BF16, kind="ExternalInput").ap()
    wq = dt("wq", (D, F), BF16, kind="ExternalInput").ap()
    wk = dt("wk", (D, F), BF16, kind="ExternalInput").ap()
    wv = dt("wv", (D, F), BF16, kind="ExternalInput").ap()
    wout = dt("wout", (F, D), BF16, kind="ExternalInput").ap()
    cq = dt("cq", (128, L), BF16, kind="ExternalInput").ap()
    sq_t = dt("sq", (128, L), BF16, kind="ExternalInput").ap()
    ck = dt("ck", (128, L), BF16, kind="ExternalInput").ap()
    sk_t = dt("sk", (128, L), BF16, kind="ExternalInput").ap()
    bdiag_d = dt("bdiag", (128, 2), BF16, kind="ExternalInput").ap()
    bmap_d = dt("bmap", (2, 128), BF16, kind="ExternalInput").ap()
    perm_d = dt("perm", (128, 128), BF16, kind="ExternalInput").ap()
    selA_d = dt("selA", (128, 128), BF16, kind="ExternalInput").ap()
    selB_d = dt("selB", (128, 128), BF16, kind="ExternalInput").ap()
    outT = dt("outT", (D, L), F32, kind="ExternalOutput").ap()

    with tile.TileContext(nc) as tc:
        with ExitStack() as ctx:
            big = ctx.enter_context(tc.tile_pool(name="big", bufs=1))
            dram = ctx.enter_context(tc.tile_pool(name="dram", bufs=1, space="DRAM"))

            kT = [big.tile([128, L], BF16, tag=f"kT{i}", name=f"kT{i}") for i in range(4)]
            qT = [big.tile([128, L], BF16, tag=f"qT{i}", name=f"qT{i}") for i in range(4)]
            vaug = [big.tile([128, HC * 65], BF16, tag=f"v{i}", name=f"vaug{i}") for i in range(16)]
            rk_dram = dram.tile([HC, L], F32, tag="rk")

            bdiag = big.tile([128, 2], BF16, tag="bdiag")
            nc.sync.dma_start(bdiag[:], bdiag_d[:])
            bmap = big.tile([2, 128], BF16, tag="bmap")
            nc.sync.dma_start(bmap[:], bmap_d[:])
            perm = big.tile([128, 128], BF16, tag="perm")
            nc.sync.dma_start(perm[:], perm_d[:])

            # ---- Phase A: kv projections (k feature-major + v into vaug) ----
            with ExitStack() as actx:
                _proj_side(tc, actx, kT, xkvT, wk, ck, sk_t, bdiag, bmap, perm,
                           side="k", rk_dram=rk_dram, wv_dram=wv, vaug=vaug)

            # ---- Phase B: q projection ----
            with ExitStack() as bctx:
                _proj_side(tc, bctx, qT, xqT, wq, cq, sq_t, bdiag, bmap, perm,
                           side="q")

            # ---- Phases C+D persistents ----
            p2 = ctx.enter_context(tc.tile_pool(name="p2", bufs=1))
            ytr = [p2.tile([128, L], BF16, tag=f"ytr{i}", name=f"ytr{i}")
                   for i in range(4)]
            sums_g = [p2.tile([128, L], F32, tag=f"sums{g}", name=f"sums{g}")
                      for g in range(2)]
            nc.gpsimd.memset(sums_g[0][:], 1.0)
            nc.gpsimd.memset(sums_g[1][:], 1.0)

            # ---- Phase C: attention ----
            with ExitStack() as cctx:
                cpool = cctx.enter_context(tc.tile_pool(name="att_sb", bufs=1))
                cps = cctx.enter_context(
                    tc.tile_pool(name="att_ps", bufs=1, space="PSUM"))
                rk_sb = cpool.tile([128, HC, 16], F32, tag="rk")
                nc.sync.dma_start(
                    rk_sb[:], rk_dram.rearrange("h (kc p) -> p h kc", p=128))
                va3s = [vaug[kc].rearrange("p (h c) -> p h c", c=65)
                        for kc in range(16)]
                for h in range(HC):
                    fb, off = h // 2, (h % 2) * 64
                    yps = [cps.tile([65, 512], F32, tag=f"y{qn}", bufs=1,
                                    name=f"yps{h}_{qn}")
                           for qn in range(4)]
                    pend = None   # (kc, [pt_half0, pt_half1]) awaiting attnv
                    for kc in range(16):
                        pts = []
                        for half in range(2):
                            sps = cps.tile([128, 1024], F32, tag="s", bufs=2)
                            for j in range(2):
                                qn = half * 2 + j
                                nc.tensor.matmul(
                                    sps[:, j * 512:(j + 1) * 512],
                                    kT[fb][off:off + 64,
                                           kc * 128:(kc + 1) * 128],
                                    qT[fb][off:off + 64,
                                           qn * 512:(qn + 1) * 512],
                                    start=True, stop=True)
                            pt = cpool.tile([128, 1024], BF16, tag="p", bufs=4)
                            nc.scalar.activation(pt[:], sps[:], EXP,
                                                 scale=rk_sb[:, h, kc:kc + 1])
                            pts.append(pt)
                        if pend is not None:
                            pkc, ppts = pend
                            for half in range(2):
                                for j in range(2):
                                    qn = half * 2 + j
                                    nc.tensor.matmul(
                                        yps[qn][:], va3s[pkc][:, h, :],
                                        ppts[half][:, j * 512:(j + 1) * 512],
                                        start=(pkc == 0), stop=False)
                        pend = (kc, pts)
                    pkc, ppts = pend
                    for half in range(2):
                        for j in range(2):
                            qn = half * 2 + j
                            nc.tensor.matmul(
                                yps[qn][:], va3s[pkc][:, h, :],
                                ppts[half][:, j * 512:(j + 1) * 512],
                                start=False, stop=True)
                    for qn in range(4):
                        nc.vector.tensor_copy(
                            ytr[fb][off:off + 64, qn * 512:(qn + 1) * 512],
                            yps[qn][0:64, :])
                        slot = 32 * (h % 4)
                        nc.vector.tensor_copy(
                            sums_g[h // 4][slot:slot + 1,
                                           qn * 512:(qn + 1) * 512],
                            yps[qn][64:65, :])

            # ---- Phase D: normalize + output projection ----
            with ExitStack() as dctx:
                dpool = dctx.enter_context(tc.tile_pool(name="out_sb", bufs=1))
                dps = dctx.enter_context(
                    tc.tile_pool(name="out_ps", bufs=1, space="PSUM"))
                wo_sb = []
                for fc in range(4):
                    w = dpool.tile([128, D], BF16, tag=f"wo{fc}")
                    nc.sync.dma_start(w[:], wout[fc * 128:(fc + 1) * 128, :])
                    wo_sb.append(w)
                sel_sb = []
                for i, sd in enumerate((selA_d, selB_d)):
                    s = dpool.tile([128, 128], BF16, tag=f"sel{i}",
                                   name=f"sel{i}")
                    nc.sync.dma_start(s[:], sd[:])
                    sel_sb.append(s)
                rs_g = []
                for g in range(2):
                    rs32 = dpool.tile([128, L], F32, tag="rs32", bufs=2,
                                      name=f"rs32_{g}")
                    nc.vector.reciprocal_approx_fast(
                        out=rs32[:], in_=sums_g[g][:])
                    rs = dpool.tile([128, L], BF16, tag="rs", bufs=2,
                                    name=f"rs{g}")
                    nc.vector.tensor_copy(rs[:], rs32[:])
                    rs_g.append(rs)
                for fb in range(4):
                    sel = sel_sb[fb % 2]
                    bps = dps.tile([128, L], F32, tag="bc2", bufs=1)
                    for qn in range(4):
                        nc.tensor.matmul(
                            bps[:, qn * 512:(qn + 1) * 512],
                            sel[:],
                            rs_g[fb // 2][:, qn * 512:(qn + 1) * 512],
                            start=True, stop=True)
                    nc.vector.tensor_mul(ytr[fb][:], ytr[fb][:], bps[:])
                for nb in range(8):
                    for qn in range(4):
                        ps = dps.tile([128, 512], F32, tag="oproj", bufs=3)
                        for fc in range(4):
                            nc.tensor.matmul(
                                ps[:],
                                wo_sb[fc][:, nb * 128:(nb + 1) * 128],
                                ytr[fc][:, qn * 512:(qn + 1) * 512],
                                start=(fc == 0), stop=(fc == 3))
                        ot = dpool.tile([128, 512], F32, tag="ot", bufs=3)
                        nc.vector.tensor_copy(ot[:], ps[:])
                        nc.sync.dma_start(
                            outT[nb * 128:(nb + 1) * 128,
                                 qn * 512:(qn + 1) * 512], ot[:])
    nc.compile()
    return nc


def get_nc():
    global _NC
    if _NC is None:
        _NC = _build_program()
    return _NC


# --------------------------------------------------------------------------- #
# Host side
# --------------------------------------------------------------------------- #

def _rope_tables(pos, g):
    """Feature-major folded RoPE(+gain) tables, replicated for a 2-head tile."""
    pos = np.asarray(pos).astype(np.float32)
    g = np.asarray(g, dtype=np.float32)
    inv = (1.0 / (10000.0 ** (np.arange(0, DH, 2, dtype=np.float32)
                              / np.float32(DH)))).astype(np.float32)
    ang = pos[:, None] * inv[None, :]                      # (L, 32)
    cos, sin = np.cos(ang, dtype=np.float32), np.sin(ang, dtype=np.float32)
    j = np.arange(DH)
    C = (g[j][:, None] * cos[:, j % 32].T).astype(np.float32)       # (64, L)
    sign = np.where(j < 32, -1.0, 1.0).astype(np.float32)
    S = (sign[:, None] * g[(j + 32) % 64][:, None]
         * sin[:, j % 32].T).astype(np.float32)
    return (np.ascontiguousarray(np.tile(C, (2, 1))).astype(NP_BF16),
            np.ascontiguousarray(np.tile(S, (2, 1))).astype(NP_BF16))


def make_in_maps(queries, kv, Wq, Wkv, Wout, g_q, g_k, pos_q, pos_k):
    queries = np.asarray(queries, dtype=np.float32)
    kv = np.asarray(kv, dtype=np.float32)
    Wq = np.asarray(Wq, dtype=np.float32)
    Wkv = np.asarray(Wkv, dtype=np.float32)
    Wout = np.asarray(Wout, dtype=np.float32)

    cq, sq = _rope_tables(pos_q, g_q)
    ck, sk = _rope_tables(pos_k, g_k)
    bdiag = np.zeros((128, 2), np.float32)
    bdiag[0:64, 0] = 1.0
    bdiag[64:128, 1] = 1.0
    bmap = np.zeros((2, 128), np.float32)
    bmap[0, 0:64] = 1.0
    bmap[1, 64:128] = 1.0
    # unsigned rotate-half permutation (sign lives in the S table):
    # rot[i] = raw[i+32] for i%64<32 else raw[i-32]
    perm = np.zeros((128, 128), np.float32)
    for i in range(128):
        src = i + 32 if (i % 64) < 32 else i - 32
        perm[src, i] = 1.0
    selA = np.zeros((128, 128), np.float32)
    selA[0, 0:64] = 1.0
    selA[32, 64:128] = 1.0
    selB = np.zeros((128, 128), np.float32)
    selB[64, 0:64] = 1.0
    selB[96, 64:128] = 1.0

    Wkv3 = Wkv.reshape(D, 16, 2 * DH)
    in_maps = []
    for c in range(N_CORES):
        b, grp = c // 2, c % 2
        hs = slice(grp * HC, (grp + 1) * HC)
        in_maps.append({
            "xqT": np.ascontiguousarray(queries[b].T).astype(NP_BF16),
            "xkvT": np.ascontiguousarray(kv[b].T).astype(NP_BF16),
            "wq": np.ascontiguousarray(
                Wq[:, grp * F:(grp + 1) * F]).astype(NP_BF16),
            "wk": np.ascontiguousarray(
                Wkv3[:, hs, :DH].reshape(D, F)).astype(NP_BF16),
            "wv": np.ascontiguousarray(
                Wkv3[:, hs, DH:].reshape(D, F)).astype(NP_BF16),
            "wout": np.ascontiguousarray(
                Wout[grp * F:(grp + 1) * F, :]).astype(NP_BF16),
            "cq": cq, "sq": sq, "ck": ck, "sk": sk,
            "bdiag": bdiag.astype(NP_BF16), "bmap": bmap.astype(NP_BF16),
            "perm": perm.astype(NP_BF16),
            "selA": selA.astype(NP_BF16), "selB": selB.astype(NP_BF16),
        })
    return in_maps


def kernel(queries, kv, Wq, Wkv, Wout, g_q, g_k, pos_q, pos_k):
    global LAST_RESULTS
    nc = get_nc()
    in_maps = make_in_maps(queries, kv, Wq, Wkv, Wout, g_q, g_k, pos_q, pos_k)
    trace = bool(int(os.environ.get("KERNEL_TRACE", "0")))
    kw = {}
    if trace:
        kw["tmpdir"] = os.environ.get("KERNEL_TRACE_DIR") or None
    res = run_bass_kernel_spmd(nc, in_maps, core_ids=list(range(N_CORES)),
                               trace=trace, **kw)
    LAST_RESULTS = res
    out = np.empty((4, L, D), np.float32)
    for b in range(4):
        out[b] = (res.results[2 * b]["outT"]
                  + res.results[2 * b + 1]["outT"]).T
    return out


# revision 6
# speedup vs baseline: 1.8947x; 1.1825x over previous
"""Trainium2 Bass kernel for nn_CrossAttention (B=4, Lq=Lk=2048, D=1024, H=16, d=64).

Sharding: 8 cores = 4 batches x 2 head-groups (8 heads each).
Each core computes a partial out^T = Wout_g^T @ y_g^T for its (batch, head-group);
host sums the two head-group partials per batch and transposes.

v2: all matmuls run in bf16 (f32r matmuls cost ~2x bf16 on HW), and the RoPE
rotate-half moved from GpSimd partition copies onto the PE as a permutation
matmul. Projection phases are software-pipelined (proj c | perm/var c-1 |
bcast c-2) so the PE never waits on the vector/scalar RMSNorm chain.

Device layout is feature-major ("T" = [feature, seq]) throughout:
  qT/kT: [512, L] bf16 (8 heads x 64 dims on partitions, seq on free axis)
  S^T:   [k, q] tiles -> softmax sum via an appended ones-column in v (M=65)
  exp:   ACT, with the k-side RMSNorm rstd (and the 1/sqrt(d) scale) folded
         into the per-partition activation scale operand.
"""
import os
import numpy as np
from contextlib import ExitStack

import concourse.bass as bass
import concourse.tile as tile
from concourse import bacc, mybir
from concourse.bass_utils import run_bass_kernel_spmd

F32 = mybir.dt.float32
BF16 = mybir.dt.bfloat16
NP_BF16 = mybir.dt.np(BF16)
EXP = mybir.ActivationFunctionType.Exp
SQUARE = mybir.ActivationFunctionType.Square
SQRT = mybir.ActivationFunctionType.Sqrt
COPYF = mybir.ActivationFunctionType.Copy

D = 1024          # model dim
L = 2048          # seq len (q and k)
HC = 8            # heads per core
DH = 64           # head dim
F = HC * DH       # 512 local features
N_CORES = 8
EPS = float(np.finfo(np.float32).eps)

LAST_RESULTS = None  # BassKernelResults of the most recent run (for test harness)
_NC = None


# --------------------------------------------------------------------------- #
# Device program
# --------------------------------------------------------------------------- #

def _proj_side(tc, ctx, dst, x_dram, w_dram, c_dram, s_dram, bdiag, bmap, perm,
               side, rk_dram=None, wv_dram=None, vaug=None):
    """Project x (via w) into feature-major bf16 dst tiles [128, L] x4, with
    RMSNorm + RoPE applied. Software-pipelined over 16 chunks [128, 512].

    side == "q": multiply rstd into dst (via broadcast matmul).
    side == "k": write 0.125*rstd chunks to rk_dram instead (consumed by exp),
                 and also project v (wv_dram) into vaug tiles.
    """
    nc = tc.nc
    pool = ctx.enter_context(tc.tile_pool(name=f"{side}_sb", bufs=1))
    pps = ctx.enter_context(tc.tile_pool(name=f"{side}_ps", bufs=1, space="PSUM"))

    # full input first (the first matmuls gate on x0/w0), tables last
    x_sb = []
    for dc in range(8):
        x = pool.tile([128, L], BF16, tag=f"x{dc}")
        nc.sync.dma_start(x[:], x_dram[dc * 128:(dc + 1) * 128, :])
        x_sb.append(x)
    # weights [128, F] x8 bf16
    w_sb = []
    for dc in range(8):
        w = pool.tile([128, F], BF16, tag=f"w{dc}")
        nc.sync.dma_start(w[:], w_dram[dc * 128:(dc + 1) * 128, :])
        w_sb.append(w)
    wv_sb = []
    if wv_dram is not None:
        for dc in range(8):
            w = pool.tile([128, F], BF16, tag=f"wv{dc}", name=f"wv_sb{dc}")
            nc.sync.dma_start(w[:], wv_dram[dc * 128:(dc + 1) * 128, :])
            wv_sb.append(w)
    # rope tables [128, L] bf16 (only needed from stage2 on)
    c_sb = pool.tile([128, L], BF16, tag="ctab")
    nc.sync.dma_start(c_sb[:], c_dram[:])
    s_sb = pool.tile([128, L], BF16, tag="stab")
    nc.sync.dma_start(s_sb[:], s_dram[:])
    eps_t = pool.tile([2, 1], F32, tag="eps", name=f"eps_{side}")
    nc.gpsimd.memset(eps_t[:], EPS if side == "q" else 64.0 * EPS)

    chunks = [(fb, qc) for fb in range(4) for qc in range(4)]
    st = [dict() for _ in chunks]   # per-chunk pipeline state

    def stage1(c):
        fb, qc = chunks[c]
        col0 = qc * 512
        ps = pps.tile([128, 512], F32, tag="proj", bufs=2)
        for dc in range(8):
            nc.tensor.matmul(ps[:],
                             w_sb[dc][:, fb * 128:(fb + 1) * 128],
                             x_sb[dc][:, col0:col0 + 512],
                             start=(dc == 0), stop=(dc == 7))
        raw = pool.tile([128, 512], BF16, tag="raw", bufs=3)
        nc.vector.tensor_copy(raw[:], ps[:])          # cast for perm matmul
        sq = pool.tile([128, 512], BF16, tag="sq", bufs=2)
        nc.scalar.activation(sq[:], ps[:], SQUARE)
        t1 = pool.tile([128, 512], BF16, tag="t1", bufs=3)
        nc.gpsimd.tensor_mul(t1[:], raw[:], c_sb[:, col0:col0 + 512])
        st[c].update(ps=ps, raw=raw, sq=sq, t1=t1, col0=col0, fb=fb)

    def stage1v(kc):
        # v projection chunk kc -> vaug[kc] (seq-major), k side only
        ps = pps.tile([128, 512], F32, tag="vproj", bufs=2)
        for dc in range(8):
            nc.tensor.matmul(ps[:],
                             x_sb[dc][:, kc * 128:(kc + 1) * 128],
                             wv_sb[dc][:],
                             start=(dc == 0), stop=(dc == 7))
        va = vaug[kc]
        nc.gpsimd.memset(va[:], 1.0)
        va3 = va.rearrange("p (h c) -> p h c", c=65)
        ps3 = ps.rearrange("p (h c) -> p h c", c=64)
        nc.vector.tensor_copy(va3[:, :, 0:64], ps3[:])

    def stage2(c):
        s = st[c]
        fb, col0 = s["fb"], s["col0"]
        rot = pps.tile([128, 512], F32, tag="rot", bufs=2)
        nc.tensor.matmul(rot[:], perm[:], s["raw"][:], start=True, stop=True)
        vps = pps.tile([2, 512], F32, tag="var", bufs=2)
        nc.tensor.matmul(vps[:], bdiag[:], s["sq"][:], start=True, stop=True)
        std = pool.tile([2, 512], F32, tag="std", bufs=2)
        if side == "q":
            # std = sqrt(raw/64 + eps); rstd = 1/std
            nc.scalar.activation(std[:], vps[:], SQRT,
                                 bias=eps_t[:], scale=1.0 / 64.0)
        else:
            # fold the 1/8 attention scale: rk = 1/(8*std) = 1/sqrt(64*(raw/64+eps))
            nc.scalar.activation(std[:], vps[:], SQRT,
                                 bias=eps_t[:], scale=1.0)
        t2 = pool.tile([128, 512], BF16, tag="t2", bufs=2)
        nc.vector.tensor_mul(t2[:], rot[:], s_sb[:, col0:col0 + 512])
        if side == "q":
            rstd = pool.tile([2, 512], F32, tag="rstd", bufs=3)
            nc.vector.reciprocal_approx_fast(out=rstd[:], in_=std[:])
            rstd_b = pool.tile([2, 512], BF16, tag="rstdb", bufs=3)
            nc.scalar.activation(rstd_b[:], rstd[:], COPYF)
            pre = pool.tile([128, 512], BF16, tag="pre", bufs=3)
            nc.gpsimd.tensor_add(pre[:], s["t1"][:], t2[:])
            s.update(rstd_b=rstd_b, pre=pre)
        else:
            rstd = pool.tile([2, 512], F32, tag="rstd", bufs=2)
            nc.vector.reciprocal_approx_fast(out=rstd[:], in_=std[:])
            nc.sync.dma_start(
                rk_dram[2 * fb:2 * fb + 2, col0:col0 + 512], rstd[:])
            nc.vector.tensor_add(dst[fb][:, col0:col0 + 512], s["t1"][:], t2[:])

    def stage3(c):
        # q only: broadcast rstd over the 2x64 head rows and multiply in
        s = st[c]
        fb, col0 = s["fb"], s["col0"]
        bps = pps.tile([128, 512], F32, tag="bc", bufs=2)
        nc.tensor.matmul(bps[:], bmap[:], s["rstd_b"][:], start=True, stop=True)
        nc.vector.tensor_mul(dst[fb][:, col0:col0 + 512], s["pre"][:], bps[:])
        st[c] = {}

    n = len(chunks)
    if side == "k":
        for i in range(n + 1):
            if i < n:
                stage1(i)
                stage1v(i)
            if i >= 1:
                stage2(i - 1)
    else:
        for i in range(n + 2):
            if i < n:
                stage1(i)
            if 1 <= i <= n:
                stage2(i - 1)
            if i >= 2:
                stage3(i - 2)


def _build_program():
    nc = bacc.Bacc("TRN2", target_bir_lowering=False, debug=False,
                   num_devices=N_CORES)
    dt = nc.dram_tensor
    xqT = dt("xqT", (D, L), BF16, kind="ExternalInput").ap()
    xkvT = dt("xkvT", (D, L), BF16, kind="ExternalInput").ap()
    wq = dt("wq", (D, F), BF16, kind="ExternalInput").ap()
    wk = dt("wk", (D, F), BF16, kind="ExternalInput").ap()
    wv = dt("wv", (D, F), BF16, kind="ExternalInput").ap()
    wout = dt("wout", (F, D), BF16, kind="ExternalInput").ap()
    cq = dt("cq", (128, L), BF16, kind="ExternalInput").ap()
    sq_t = dt("sq", (128, L), BF16, kind="ExternalInput").ap()
    ck = dt("ck", (128, L), BF16, kind="ExternalInput").ap()
    sk_t = dt("sk", (128, L), BF16, kind="ExternalInput").ap()
    bdiag_d = dt("bdiag", (128, 2), BF16, kind="ExternalInput").ap()
    bmap_d = dt("bmap", (2, 128), BF16, kind="ExternalInput").ap()
    perm_d = dt("perm", (128, 128), BF16, kind="ExternalInput").ap()
    selA_d = dt("selA", (128, 128), BF16, kind="ExternalInput").ap()
    selB_d = dt("selB", (128, 128), BF16, kind="ExternalInput").ap()
    outT = dt("outT", (D, L), F32, kind="ExternalOutput").ap()

    with tile.TileContext(nc) as tc:
        with ExitStack() as ctx:
            big = ctx.enter_context(tc.tile_pool(name="big", bufs=1))
            dram = ctx.enter_context(tc.tile_pool(name="dram", bufs=1, space="DRAM"))

            kT = [big.tile([128, L], BF16, tag=f"kT{i}", name=f"kT{i}") for i in range(4)]
            qT = [big.tile([128, L], BF16, tag=f"qT{i}", name=f"qT{i}") for i in range(4)]
            vaug = [big.tile([128, HC * 65], BF16, tag=f"v{i}", name=f"vaug{i}") for i in range(16)]
            rk_dram = dram.tile([HC, L], F32, tag="rk")

            bdiag = big.tile([128, 2], BF16, tag="bdiag")
            nc.sync.dma_start(bdiag[:], bdiag_d[:])
            bmap = big.tile([2, 128], BF16, tag="bmap")
            nc.sync.dma_start(bmap[:], bmap_d[:])
            perm = big.tile([128, 128], BF16, tag="perm")
            nc.sync.dma_start(perm[:], perm_d[:])

            # ---- Phase A: kv projections (k feature-major + v into vaug) ----
            with ExitStack() as actx:
                _proj_side(tc, actx, kT, xkvT, wk, ck, sk_t, bdiag, bmap, perm,
                           side="k", rk_dram=rk_dram, wv_dram=wv, vaug=vaug)

            # ---- Phases C+D persistents ----
            p2 = ctx.enter_context(tc.tile_pool(name="p2", bufs=1))
            ytr = [p2.tile([128, L], BF16, tag=f"ytr{i}", name=f"ytr{i}")
                   for i in range(4)]
            sums_g = [p2.tile([128, L], F32, tag=f"sums{g}", name=f"sums{g}")
                      for g in range(2)]
            nc.gpsimd.memset(sums_g[0][:], 1.0)
            nc.gpsimd.memset(sums_g[1][:], 1.0)
            # rk transpose gather: issue before phase B so it lands well
            # before the first exp of phase C
            rk_sb = p2.tile([128, HC, 16], F32, tag="rk")
            nc.sync.dma_start(
                rk_sb[:], rk_dram.rearrange("h (kc p) -> p h kc", p=128))
            # phase D weights, prefetched during B/C
            wo_sb = []
            for fc in range(4):
                w = p2.tile([128, D], BF16, tag=f"wo{fc}", name=f"wo{fc}")
                nc.sync.dma_start(w[:], wout[fc * 128:(fc + 1) * 128, :])
                wo_sb.append(w)
            sel_sb = []
            for i, sd in enumerate((selA_d, selB_d)):
                s = p2.tile([128, 128], BF16, tag=f"sel{i}", name=f"sel{i}")
                nc.sync.dma_start(s[:], sd[:])
                sel_sb.append(s)
            rs_g = [None, None]

            # ---- Phase B: q projection ----
            with ExitStack() as bctx:
                _proj_side(tc, bctx, qT, xqT, wq, cq, sq_t, bdiag, bmap, perm,
                           side="q")

            # ---- Phase C: attention ----
            with ExitStack() as cctx:
                cpool = cctx.enter_context(tc.tile_pool(name="att_sb", bufs=1))
                cps = cctx.enter_context(
                    tc.tile_pool(name="att_ps", bufs=1, space="PSUM"))
                va3s = [vaug[kc].rearrange("p (h c) -> p h c", c=65)
                        for kc in range(16)]
                for h in range(HC):
                    fb, off = h // 2, (h % 2) * 64
                    yps = [cps.tile([65, 512], F32, tag=f"y{qn}", bufs=1,
                                    name=f"yps{h}_{qn}")
                           for qn in range(4)]
                    pend = None   # (kc, [pt_half0, pt_half1]) awaiting attnv
                    for kc in range(16):
                        pts = []
                        for half in range(2):
                            sps = cps.tile([128, 1024], F32, tag="s", bufs=2)
                            for j in range(2):
                                qn = half * 2 + j
                                nc.tensor.matmul(
                                    sps[:, j * 512:(j + 1) * 512],
                                    kT[fb][off:off + 64,
                                           kc * 128:(kc + 1) * 128],
                                    qT[fb][off:off + 64,
                                           qn * 512:(qn + 1) * 512],
                                    start=True, stop=True)
                            pt = cpool.tile([128, 1024], BF16, tag="p", bufs=4)
                            nc.scalar.activation(pt[:], sps[:], EXP,
                                                 scale=rk_sb[:, h, kc:kc + 1])
                            pts.append(pt)
                        if pend is not None:
                            pkc, ppts = pend
                            for half in range(2):
                                for j in range(2):
                                    qn = half * 2 + j
                                    nc.tensor.matmul(
                                        yps[qn][:], va3s[pkc][:, h, :],
                                        ppts[half][:, j * 512:(j + 1) * 512],
                                        start=(pkc == 0), stop=False)
                        pend = (kc, pts)
                    pkc, ppts = pend
                    for half in range(2):
                        for j in range(2):
                            qn = half * 2 + j
                            nc.tensor.matmul(
                                yps[qn][:], va3s[pkc][:, h, :],
                                ppts[half][:, j * 512:(j + 1) * 512],
                                start=False, stop=True)
                    for qn in range(4):
                        nc.vector.tensor_copy(
                            ytr[fb][off:off + 64, qn * 512:(qn + 1) * 512],
                            yps[qn][0:64, :])
                        slot = 32 * (h % 4)
                        nc.vector.tensor_copy(
                            sums_g[h // 4][slot:slot + 1,
                                           qn * 512:(qn + 1) * 512],
                            yps[qn][64:65, :])
                    if h in (3, 7):
                        # group done: fold sums -> 1/sums on the (idle)
                        # vector engine while attention continues
                        g = h // 4
                        rs32 = p2.tile([128, L], F32, tag="rs32", bufs=2,
                                       name=f"rs32_{g}")
                        nc.vector.reciprocal_approx_fast(
                            out=rs32[:], in_=sums_g[g][:])
                        rs = p2.tile([128, L], BF16, tag="rs", bufs=2,
                                     name=f"rs{g}")
                        nc.vector.tensor_copy(rs[:], rs32[:])
                        rs_g[g] = rs

            # ---- Phase D: normalize + output projection (per-qn pipeline) --
            with ExitStack() as dctx:
                dps = dctx.enter_context(
                    tc.tile_pool(name="out_ps", bufs=1, space="PSUM"))
                dpool = dctx.enter_context(tc.tile_pool(name="out_sb", bufs=1))

                def d_stage1(qn):
                    sl = slice(qn * 512, (qn + 1) * 512)
                    for fb in range(4):
                        bps = dps.tile([128, 512], F32, tag="bc2", bufs=4,
                                       name=f"bc2_{fb}_{qn}")
                        nc.tensor.matmul(bps[:], sel_sb[fb % 2][:],
                                         rs_g[fb // 2][:, sl],
                                         start=True, stop=True)
                        nc.vector.tensor_mul(ytr[fb][:, sl], ytr[fb][:, sl],
                                             bps[:])

                def d_stage2(qn):
                    sl = slice(qn * 512, (qn + 1) * 512)
                    for nb in range(8):
                        ps = dps.tile([128, 512], F32, tag="oproj", bufs=3)
                        for fc in range(4):
                            nc.tensor.matmul(
                                ps[:],
                                wo_sb[fc][:, nb * 128:(nb + 1) * 128],
                                ytr[fc][:, sl],
                                start=(fc == 0), stop=(fc == 3))
                        ot = dpool.tile([128, 512], F32, tag="ot", bufs=3)
                        nc.scalar.activation(ot[:], ps[:], COPYF)
                        nc.sync.dma_start(
                            outT[nb * 128:(nb + 1) * 128, sl], ot[:])

                for i in range(5):
                    if i < 4:
                        d_stage1(i)
                    if i >= 1:
                        d_stage2(i - 1)
    nc.compile()
    return nc


def get_nc():
    global _NC
    if _NC is None:
        _NC = _build_program()
    return _NC


# --------------------------------------------------------------------------- #
# Host side
# --------------------------------------------------------------------------- #

def _rope_tables(pos, g):
    """Feature-major folded RoPE(+gain) tables, replicated for a 2-head tile."""
    pos = np.asarray(pos).astype(np.float32)
    g = np.asarray(g, dtype=np.float32)
    inv = (1.0 / (10000.0 ** (np.arange(0, DH, 2, dtype=np.float32)
                              / np.float32(DH)))).astype(np.float32)
    ang = pos[:, None] * inv[None, :]                      # (L, 32)
    cos, sin = np.cos(ang, dtype=np.float32), np.sin(ang, dtype=np.float32)
    j = np.arange(DH)
    C = (g[j][:, None] * cos[:, j % 32].T).astype(np.float32)       # (64, L)
    sign = np.where(j < 32, -1.0, 1.0).astype(np.float32)
    S = (sign[:, None] * g[(j + 32) % 64][:, None]
         * sin[:, j % 32].T).astype(np.float32)
    return (np.ascontiguousarray(np.tile(C, (2, 1))).astype(NP_BF16),
            np.ascontiguousarray(np.tile(S, (2, 1))).astype(NP_BF16))


def make_in_maps(queries, kv, Wq, Wkv, Wout, g_q, g_k, pos_q, pos_k):
    queries = np.asarray(queries, dtype=np.float32)
    kv = np.asarray(kv, dtype=np.float32)
    Wq = np.asarray(Wq, dtype=np.float32)
    Wkv = np.asarray(Wkv, dtype=np.float32)
    Wout = np.asarray(Wout, dtype=np.float32)

    cq, sq = _rope_tables(pos_q, g_q)
    ck, sk = _rope_tables(pos_k, g_k)
    bdiag = np.zeros((128, 2), np.float32)
    bdiag[0:64, 0] = 1.0
    bdiag[64:128, 1] = 1.0
    bmap = np.zeros((2, 128), np.float32)
    bmap[0, 0:64] = 1.0
    bmap[1, 64:128] = 1.0
    # unsigned rotate-half permutation (sign lives in the S table):
    # rot[i] = raw[i+32] for i%64<32 else raw[i-32]
    perm = np.zeros((128, 128), np.float32)
    for i in range(128):
        src = i + 32 if (i % 64) < 32 else i - 32
        perm[src, i] = 1.0
    selA = np.zeros((128, 128), np.float32)
    selA[0, 0:64] = 1.0
    selA[32, 64:128] = 1.0
    selB = np.zeros((128, 128), np.float32)
    selB[64, 0:64] = 1.0
    selB[96, 64:128] = 1.0

    Wkv3 = Wkv.reshape(D, 16, 2 * DH)
    in_maps = []
    for c in range(N_CORES):
        b, grp = c // 2, c % 2
        hs = slice(grp * HC, (grp + 1) * HC)
        in_maps.append({
            "xqT": np.ascontiguousarray(queries[b].T).astype(NP_BF16),
            "xkvT": np.ascontiguousarray(kv[b].T).astype(NP_BF16),
            "wq": np.ascontiguousarray(
                Wq[:, grp * F:(grp + 1) * F]).astype(NP_BF16),
            "wk": np.ascontiguousarray(
                Wkv3[:, hs, :DH].reshape(D, F)).astype(NP_BF16),
            "wv": np.ascontiguousarray(
                Wkv3[:, hs, DH:].reshape(D, F)).astype(NP_BF16),
            "wout": np.ascontiguousarray(
                Wout[grp * F:(grp + 1) * F, :]).astype(NP_BF16),
            "cq": cq, "sq": sq, "ck": ck, "sk": sk,
            "bdiag": bdiag.astype(NP_BF16), "bmap": bmap.astype(NP_BF16),
            "perm": perm.astype(NP_BF16),
            "selA": selA.astype(NP_BF16), "selB": selB.astype(NP_BF16),
        })
    return in_maps


def kernel(queries, kv, Wq, Wkv, Wout, g_q, g_k, pos_q, pos_k):
    global LAST_RESULTS
    nc = get_nc()
    in_maps = make_in_maps(queries, kv, Wq, Wkv, Wout, g_q, g_k, pos_q, pos_k)
    trace = bool(int(os.environ.get("KERNEL_TRACE", "0")))
    kw = {}
    if trace:
        kw["tmpdir"] = os.environ.get("KERNEL_TRACE_DIR") or None
    res = run_bass_kernel_spmd(nc, in_maps, core_ids=list(range(N_CORES)),
                               trace=trace, **kw)
    LAST_RESULTS = res
    out = np.empty((4, L, D), np.float32)
    for b in range(4):
        out[b] = (res.results[2 * b]["outT"]
                  + res.results[2 * b + 1]["outT"]).T
    return out


# revision 13
# speedup vs baseline: 1.9165x; 1.0115x over previous
"""Trainium2 Bass kernel for nn_CrossAttention (B=4, Lq=Lk=2048, D=1024, H=16, d=64).

Sharding: 8 cores = 4 batches x 2 head-groups (8 heads each).
Each core computes a partial out^T = Wout_g^T @ y_g^T for its (batch, head-group);
host sums the two head-group partials per batch and transposes.

v2: all matmuls run in bf16 (f32r matmuls cost ~2x bf16 on HW), and the RoPE
rotate-half moved from GpSimd partition copies onto the PE as a permutation
matmul. Projection phases are software-pipelined (proj c | perm/var c-1 |
bcast c-2) so the PE never waits on the vector/scalar RMSNorm chain.

Device layout is feature-major ("T" = [feature, seq]) throughout:
  qT/kT: [512, L] bf16 (8 heads x 64 dims on partitions, seq on free axis)
  S^T:   [k, q] tiles -> softmax sum via an appended ones-column in v (M=65)
  exp:   ACT, with the k-side RMSNorm rstd (and the 1/sqrt(d) scale) folded
         into the per-partition activation scale operand.
"""
import os
import numpy as np
from contextlib import ExitStack

import concourse.bass as bass
import concourse.tile as tile
from concourse import bacc, mybir
from concourse.bass_utils import run_bass_kernel_spmd

F32 = mybir.dt.float32
BF16 = mybir.dt.bfloat16
NP_BF16 = mybir.dt.np(BF16)
EXP = mybir.ActivationFunctionType.Exp
SQUARE = mybir.ActivationFunctionType.Square
SQRT = mybir.ActivationFunctionType.Sqrt
COPYF = mybir.ActivationFunctionType.Copy

D = 1024          # model dim
L = 2048          # seq len (q and k)
HC = 8            # heads per core
DH = 64           # head dim
F = HC * DH       # 512 local features
N_CORES = 8
EPS = float(np.finfo(np.float32).eps)

LAST_RESULTS = None  # BassKernelResults of the most recent run (for test harness)
_NC = None


# --------------------------------------------------------------------------- #
# Device program
# --------------------------------------------------------------------------- #

def _proj_side(tc, ctx, dst, x_dram, w_dram, c_dram, s_dram, bdiag, bmap, perm,
               side, rk_dram=None, wv_dram=None, vaug=None):
    """Project x (via w) into feature-major bf16 dst tiles [128, L] x4, with
    RMSNorm + RoPE applied. Software-pipelined over 16 chunks [128, 512].

    side == "q": multiply rstd into dst (via broadcast matmul).
    side == "k": write 0.125*rstd chunks to rk_dram instead (consumed by exp),
                 and also project v (wv_dram) into vaug tiles.
    """
    nc = tc.nc
    pool = ctx.enter_context(tc.tile_pool(name=f"{side}_sb", bufs=1))
    pps = ctx.enter_context(tc.tile_pool(name=f"{side}_ps", bufs=1, space="PSUM"))

    # full input first (the first matmuls gate on x0/w0), tables last
    x_sb = []
    for dc in range(8):
        x = pool.tile([128, L], BF16, tag=f"x{dc}")
        nc.sync.dma_start(x[:], x_dram[dc * 128:(dc + 1) * 128, :])
        x_sb.append(x)
    # weights [128, F] x8 bf16
    w_sb = []
    for dc in range(8):
        w = pool.tile([128, F], BF16, tag=f"w{dc}")
        nc.sync.dma_start(w[:], w_dram[dc * 128:(dc + 1) * 128, :])
        w_sb.append(w)
    wv_sb = []
    if wv_dram is not None:
        for dc in range(8):
            w = pool.tile([128, F], BF16, tag=f"wv{dc}", name=f"wv_sb{dc}")
            nc.sync.dma_start(w[:], wv_dram[dc * 128:(dc + 1) * 128, :])
            wv_sb.append(w)
    # rope tables [128, L] bf16 (only needed from stage2 on)
    c_sb = pool.tile([128, L], BF16, tag="ctab")
    nc.sync.dma_start(c_sb[:], c_dram[:])
    s_sb = pool.tile([128, L], BF16, tag="stab")
    nc.sync.dma_start(s_sb[:], s_dram[:])
    eps_t = pool.tile([2, 1], F32, tag="eps", name=f"eps_{side}")
    nc.gpsimd.memset(eps_t[:], EPS if side == "q" else 64.0 * EPS)

    chunks = [(fb, qc) for fb in range(4) for qc in range(4)]
    st = [dict() for _ in chunks]   # per-chunk pipeline state

    def stage1(c):
        fb, qc = chunks[c]
        col0 = qc * 512
        ps = pps.tile([128, 512], F32, tag="proj", bufs=2)
        for dc in range(8):
            nc.tensor.matmul(ps[:],
                             w_sb[dc][:, fb * 128:(fb + 1) * 128],
                             x_sb[dc][:, col0:col0 + 512],
                             start=(dc == 0), stop=(dc == 7))
        raw = pool.tile([128, 512], BF16, tag="raw", bufs=3)
        nc.vector.tensor_copy(raw[:], ps[:])          # cast for perm matmul
        sq = pool.tile([128, 512], BF16, tag="sq", bufs=2)
        nc.scalar.activation(sq[:], ps[:], SQUARE)
        t1 = pool.tile([128, 512], BF16, tag="t1", bufs=3)
        nc.gpsimd.tensor_mul(t1[:], raw[:], c_sb[:, col0:col0 + 512])
        st[c].update(ps=ps, raw=raw, sq=sq, t1=t1, col0=col0, fb=fb)

    def stage1v(kc):
        # v projection chunk kc -> vaug[kc] (seq-major), k side only
        ps = pps.tile([128, 512], F32, tag="vproj", bufs=2)
        for dc in range(8):
            nc.tensor.matmul(ps[:],
                             x_sb[dc][:, kc * 128:(kc + 1) * 128],
                             wv_sb[dc][:],
                             start=(dc == 0), stop=(dc == 7))
        va = vaug[kc]
        nc.gpsimd.memset(va[:], 1.0)
        va3 = va.rearrange("p (h c) -> p h c", c=65)
        ps3 = ps.rearrange("p (h c) -> p h c", c=64)
        nc.vector.tensor_copy(va3[:, :, 0:64], ps3[:])

    def stage2(c):
        s = st[c]
        fb, col0 = s["fb"], s["col0"]
        rot = pps.tile([128, 512], F32, tag="rot", bufs=2)
        nc.tensor.matmul(rot[:], perm[:], s["raw"][:], start=True, stop=True)
        vps = pps.tile([2, 512], F32, tag="var", bufs=2)
        nc.tensor.matmul(vps[:], bdiag[:], s["sq"][:], start=True, stop=True)
        std = pool.tile([2, 512], F32, tag="std", bufs=2)
        if side == "q":
            # std = sqrt(raw/64 + eps); rstd = 1/std
            nc.scalar.activation(std[:], vps[:], SQRT,
                                 bias=eps_t[:], scale=1.0 / 64.0)
        else:
            # fold the 1/8 attention scale: rk = 1/(8*std) = 1/sqrt(64*(raw/64+eps))
            nc.scalar.activation(std[:], vps[:], SQRT,
                                 bias=eps_t[:], scale=1.0)
        t2 = pool.tile([128, 512], BF16, tag="t2", bufs=2)
        nc.vector.tensor_mul(t2[:], rot[:], s_sb[:, col0:col0 + 512])
        if side == "q":
            rstd = pool.tile([2, 512], F32, tag="rstd", bufs=3)
            nc.vector.reciprocal_approx_fast(out=rstd[:], in_=std[:])
            rstd_b = pool.tile([2, 512], BF16, tag="rstdb", bufs=3)
            nc.scalar.activation(rstd_b[:], rstd[:], COPYF)
            pre = pool.tile([128, 512], BF16, tag="pre", bufs=3)
            nc.gpsimd.tensor_add(pre[:], s["t1"][:], t2[:])
            s.update(rstd_b=rstd_b, pre=pre)
        else:
            rstd = pool.tile([2, 512], F32, tag="rstd", bufs=2)
            nc.vector.reciprocal_approx_fast(out=rstd[:], in_=std[:])
            # issue on the gpsimd queue: a data-dependent DMA on the Sync
            # queue would head-of-line-block phase B's input loads
            nc.gpsimd.dma_start(
                rk_dram[2 * fb:2 * fb + 2, col0:col0 + 512], rstd[:])
            nc.vector.tensor_add(dst[fb][:, col0:col0 + 512], s["t1"][:], t2[:])

    def stage3(c):
        # q only: broadcast rstd over the 2x64 head rows and multiply in
        s = st[c]
        fb, col0 = s["fb"], s["col0"]
        bps = pps.tile([128, 512], F32, tag="bc", bufs=2)
        nc.tensor.matmul(bps[:], bmap[:], s["rstd_b"][:], start=True, stop=True)
        nc.vector.tensor_mul(dst[fb][:, col0:col0 + 512], s["pre"][:], bps[:])
        st[c] = {}

    n = len(chunks)
    if side == "k":
        for i in range(n + 1):
            if i < n:
                stage1(i)
                stage1v(i)
            if i >= 1:
                stage2(i - 1)
    else:
        for i in range(n + 2):
            if i < n:
                stage1(i)
            if 1 <= i <= n:
                stage2(i - 1)
            if i >= 2:
                stage3(i - 2)


def _build_program():
    nc = bacc.Bacc("TRN2", target_bir_lowering=False, debug=False,
                   num_devices=N_CORES)
    dt = nc.dram_tensor
    xqT = dt("xqT", (D, L), BF16, kind="ExternalInput").ap()
    xkvT = dt("xkvT", (D, L), BF16, kind="ExternalInput").ap()
    wq = dt("wq", (D, F), BF16, kind="ExternalInput").ap()
    wk = dt("wk", (D, F), BF16, kind="ExternalInput").ap()
    wv = dt("wv", (D, F), BF16, kind="ExternalInput").ap()
    wout = dt("wout", (F, D), BF16, kind="ExternalInput").ap()
    cq = dt("cq", (128, L), BF16, kind="ExternalInput").ap()
    sq_t = dt("sq", (128, L), BF16, kind="ExternalInput").ap()
    ck = dt("ck", (128, L), BF16, kind="ExternalInput").ap()
    sk_t = dt("sk", (128, L), BF16, kind="ExternalInput").ap()
    bdiag_d = dt("bdiag", (128, 2), BF16, kind="ExternalInput").ap()
    bmap_d = dt("bmap", (2, 128), BF16, kind="ExternalInput").ap()
    perm_d = dt("perm", (128, 128), BF16, kind="ExternalInput").ap()
    selA_d = dt("selA", (128, 128), BF16, kind="ExternalInput").ap()
    selB_d = dt("selB", (128, 128), BF16, kind="ExternalInput").ap()
    outT = dt("outT", (D, L), BF16, kind="ExternalOutput").ap()

    with tile.TileContext(nc) as tc:
        with ExitStack() as ctx:
            big = ctx.enter_context(tc.tile_pool(name="big", bufs=1))
            dram = ctx.enter_context(tc.tile_pool(name="dram", bufs=1, space="DRAM"))

            kT = [big.tile([128, L], BF16, tag=f"kT{i}", name=f"kT{i}") for i in range(4)]
            qT = [big.tile([128, L], BF16, tag=f"qT{i}", name=f"qT{i}") for i in range(4)]
            vaug = [big.tile([128, HC * 65], BF16, tag=f"v{i}", name=f"vaug{i}") for i in range(16)]
            rk_dram = dram.tile([HC, L], F32, tag="rk")

            bdiag = big.tile([128, 2], BF16, tag="bdiag")
            nc.sync.dma_start(bdiag[:], bdiag_d[:])
            bmap = big.tile([2, 128], BF16, tag="bmap")
            nc.sync.dma_start(bmap[:], bmap_d[:])
            perm = big.tile([128, 128], BF16, tag="perm")
            nc.sync.dma_start(perm[:], perm_d[:])

            # ---- Phase A: kv projections (k feature-major + v into vaug) ----
            with ExitStack() as actx:
                _proj_side(tc, actx, kT, xkvT, wk, ck, sk_t, bdiag, bmap, perm,
                           side="k", rk_dram=rk_dram, wv_dram=wv, vaug=vaug)

            # ---- Phases C+D persistents ----
            p2 = ctx.enter_context(tc.tile_pool(name="p2", bufs=1))
            ytr = [p2.tile([128, L], BF16, tag=f"ytr{i}", name=f"ytr{i}")
                   for i in range(4)]
            sums_g = [p2.tile([128, L], F32, tag=f"sums{g}", name=f"sums{g}")
                      for g in range(2)]
            nc.gpsimd.memset(sums_g[0][:], 1.0)
            nc.gpsimd.memset(sums_g[1][:], 1.0)
            rk_sb = p2.tile([128, HC, 16], F32, tag="rk")
            rs_g = [None, None]

            # ---- Phase B: q projection ----
            with ExitStack() as bctx:
                _proj_side(tc, bctx, qT, xqT, wq, cq, sq_t, bdiag, bmap, perm,
                           side="q")

            # rk transpose gather + phase D weights: issued after phase B's
            # input loads (Sync queue is in-order), ready before first exp
            nc.sync.dma_start(
                rk_sb[:], rk_dram.rearrange("h (kc p) -> p h kc", p=128))
            wo_sb = []
            for fc in range(4):
                w = p2.tile([128, D], BF16, tag=f"wo{fc}", name=f"wo{fc}")
                nc.sync.dma_start(w[:], wout[fc * 128:(fc + 1) * 128, :])
                wo_sb.append(w)
            sel_sb = []
            for i, sd in enumerate((selA_d, selB_d)):
                s = p2.tile([128, 128], BF16, tag=f"sel{i}", name=f"sel{i}")
                nc.sync.dma_start(s[:], sd[:])
                sel_sb.append(s)

            # ---- Phase C: attention ----
            with ExitStack() as cctx:
                cpool = cctx.enter_context(tc.tile_pool(name="att_sb", bufs=1))
                cps = cctx.enter_context(
                    tc.tile_pool(name="att_ps", bufs=1, space="PSUM"))
                va3s = [vaug[kc].rearrange("p (h c) -> p h c", c=65)
                        for kc in range(16)]
                for h in range(HC):
                    fb, off = h // 2, (h % 2) * 64
                    yps = [cps.tile([65, 512], F32, tag=f"y{qn}", bufs=1,
                                    name=f"yps{h}_{qn}")
                           for qn in range(4)]
                    pend = None   # (kc, [pt_half0, pt_half1]) awaiting attnv
                    for kc in range(16):
                        pts = []
                        for half in range(2):
                            sps = cps.tile([128, 1024], F32, tag="s", bufs=2)
                            for j in range(2):
                                qn = half * 2 + j
                                nc.tensor.matmul(
                                    sps[:, j * 512:(j + 1) * 512],
                                    kT[fb][off:off + 64,
                                           kc * 128:(kc + 1) * 128],
                                    qT[fb][off:off + 64,
                                           qn * 512:(qn + 1) * 512],
                                    start=True, stop=True)
                            pt = cpool.tile([128, 1024], BF16, tag="p", bufs=4)
                            nc.scalar.activation(pt[:], sps[:], EXP,
                                                 scale=rk_sb[:, h, kc:kc + 1])
                            pts.append(pt)
                        if pend is not None:
                            pkc, ppts = pend
                            for half in range(2):
                                for j in range(2):
                                    qn = half * 2 + j
                                    nc.tensor.matmul(
                                        yps[qn][:], va3s[pkc][:, h, :],
                                        ppts[half][:, j * 512:(j + 1) * 512],
                                        start=(pkc == 0), stop=False)
                        pend = (kc, pts)
                    pkc, ppts = pend
                    for half in range(2):
                        for j in range(2):
                            qn = half * 2 + j
                            nc.tensor.matmul(
                                yps[qn][:], va3s[pkc][:, h, :],
                                ppts[half][:, j * 512:(j + 1) * 512],
                                start=False, stop=True)
                    for qn in range(4):
                        nc.vector.tensor_copy(
                            ytr[fb][off:off + 64, qn * 512:(qn + 1) * 512],
                            yps[qn][0:64, :])
                        slot = 32 * (h % 4)
                        nc.vector.tensor_copy(
                            sums_g[h // 4][slot:slot + 1,
                                           qn * 512:(qn + 1) * 512],
                            yps[qn][64:65, :])
                    if h in (3, 7):
                        # group done: fold sums -> 1/sums on the (idle)
                        # vector engine while attention continues; chunked
                        # per qn so phase D's first bcast gates on 512 cols
                        g = h // 4
                        rs = p2.tile([128, L], BF16, tag="rs", bufs=2,
                                     name=f"rs{g}")
                        for qn in range(4):
                            sl = slice(qn * 512, (qn + 1) * 512)
                            rs32 = p2.tile([128, 512], F32, tag="rs32",
                                           bufs=2)
                            nc.vector.reciprocal_approx_fast(
                                out=rs32[:], in_=sums_g[g][:, sl])
                            nc.vector.tensor_copy(rs[:, sl], rs32[:])
                        rs_g[g] = rs

            # ---- Phase D: normalize + output projection (per-qn pipeline) --
            with ExitStack() as dctx:
                dps = dctx.enter_context(
                    tc.tile_pool(name="out_ps", bufs=1, space="PSUM"))
                dpool = dctx.enter_context(tc.tile_pool(name="out_sb", bufs=1))

                def d_stage1(qn):
                    sl = slice(qn * 512, (qn + 1) * 512)
                    for fb in range(4):
                        bps = dps.tile([128, 512], F32, tag="bc2", bufs=4,
                                       name=f"bc2_{fb}_{qn}")
                        nc.tensor.matmul(bps[:], sel_sb[fb % 2][:],
                                         rs_g[fb // 2][:, sl],
                                         start=True, stop=True)
                        nc.vector.tensor_mul(ytr[fb][:, sl], ytr[fb][:, sl],
                                             bps[:])

                def d_stage2(qn):
                    sl = slice(qn * 512, (qn + 1) * 512)
                    for nb in range(8):
                        ps = dps.tile([128, 512], F32, tag="oproj", bufs=3)
                        for fc in range(4):
                            nc.tensor.matmul(
                                ps[:],
                                wo_sb[fc][:, nb * 128:(nb + 1) * 128],
                                ytr[fc][:, sl],
                                start=(fc == 0), stop=(fc == 3))
                        ot = dpool.tile([128, 512], BF16, tag="ot", bufs=3)
                        nc.vector.tensor_copy(ot[:], ps[:])
                        nc.sync.dma_start(
                            outT[nb * 128:(nb + 1) * 128, sl], ot[:])

                for i in range(5):
                    if i < 4:
                        d_stage1(i)
                    if i >= 1:
                        d_stage2(i - 1)
    nc.compile()
    return nc


def get_nc():
    global _NC
    if _NC is None:
        _NC = _build_program()
    return _NC


# --------------------------------------------------------------------------- #
# Host side
# --------------------------------------------------------------------------- #

def _rope_tables(pos, g):
    """Feature-major folded RoPE(+gain) tables, replicated for a 2-head tile."""
    pos = np.asarray(pos).astype(np.float32)
    g = np.asarray(g, dtype=np.float32)
    inv = (1.0 / (10000.0 ** (np.arange(0, DH, 2, dtype=np.float32)
                              / np.float32(DH)))).astype(np.float32)
    ang = pos[:, None] * inv[None, :]                      # (L, 32)
    cos, sin = np.cos(ang, dtype=np.float32), np.sin(ang, dtype=np.float32)
    j = np.arange(DH)
    C = (g[j][:, None] * cos[:, j % 32].T).astype(np.float32)       # (64, L)
    sign = np.where(j < 32, -1.0, 1.0).astype(np.float32)
    S = (sign[:, None] * g[(j + 32) % 64][:, None]
         * sin[:, j % 32].T).astype(np.float32)
    return (np.ascontiguousarray(np.tile(C, (2, 1))).astype(NP_BF16),
            np.ascontiguousarray(np.tile(S, (2, 1))).astype(NP_BF16))


def make_in_maps(queries, kv, Wq, Wkv, Wout, g_q, g_k, pos_q, pos_k):
    queries = np.asarray(queries, dtype=np.float32)
    kv = np.asarray(kv, dtype=np.float32)
    Wq = np.asarray(Wq, dtype=np.float32)
    Wkv = np.asarray(Wkv, dtype=np.float32)
    Wout = np.asarray(Wout, dtype=np.float32)

    cq, sq = _rope_tables(pos_q, g_q)
    ck, sk = _rope_tables(pos_k, g_k)
    bdiag = np.zeros((128, 2), np.float32)
    bdiag[0:64, 0] = 1.0
    bdiag[64:128, 1] = 1.0
    bmap = np.zeros((2, 128), np.float32)
    bmap[0, 0:64] = 1.0
    bmap[1, 64:128] = 1.0
    # unsigned rotate-half permutation (sign lives in the S table):
    # rot[i] = raw[i+32] for i%64<32 else raw[i-32]
    perm = np.zeros((128, 128), np.float32)
    for i in range(128):
        src = i + 32 if (i % 64) < 32 else i - 32
        perm[src, i] = 1.0
    selA = np.zeros((128, 128), np.float32)
    selA[0, 0:64] = 1.0
    selA[32, 64:128] = 1.0
    selB = np.zeros((128, 128), np.float32)
    selB[64, 0:64] = 1.0
    selB[96, 64:128] = 1.0

    Wkv3 = Wkv.reshape(D, 16, 2 * DH)
    in_maps = []
    for c in range(N_CORES):
        b, grp = c // 2, c % 2
        hs = slice(grp * HC, (grp + 1) * HC)
        in_maps.append({
            "xqT": np.ascontiguousarray(queries[b].T).astype(NP_BF16),
            "xkvT": np.ascontiguousarray(kv[b].T).astype(NP_BF16),
            "wq": np.ascontiguousarray(
                Wq[:, grp * F:(grp + 1) * F]).astype(NP_BF16),
            "wk": np.ascontiguousarray(
                Wkv3[:, hs, :DH].reshape(D, F)).astype(NP_BF16),
            "wv": np.ascontiguousarray(
                Wkv3[:, hs, DH:].reshape(D, F)).astype(NP_BF16),
            "wout": np.ascontiguousarray(
                Wout[grp * F:(grp + 1) * F, :]).astype(NP_BF16),
            "cq": cq, "sq": sq, "ck": ck, "sk": sk,
            "bdiag": bdiag.astype(NP_BF16), "bmap": bmap.astype(NP_BF16),
            "perm": perm.astype(NP_BF16),
            "selA": selA.astype(NP_BF16), "selB": selB.astype(NP_BF16),
        })
    return in_maps


def kernel(queries, kv, Wq, Wkv, Wout, g_q, g_k, pos_q, pos_k):
    global LAST_RESULTS
    nc = get_nc()
    in_maps = make_in_maps(queries, kv, Wq, Wkv, Wout, g_q, g_k, pos_q, pos_k)
    trace = bool(int(os.environ.get("KERNEL_TRACE", "0")))
    kw = {}
    if trace:
        kw["tmpdir"] = os.environ.get("KERNEL_TRACE_DIR") or None
    res = run_bass_kernel_spmd(nc, in_maps, core_ids=list(range(N_CORES)),
                               trace=trace, **kw)
    LAST_RESULTS = res
    out = np.empty((4, L, D), np.float32)
    for b in range(4):
        out[b] = (res.results[2 * b]["outT"].astype(np.float32)
                  + res.results[2 * b + 1]["outT"].astype(np.float32)).T
    return out


# revision 30
# speedup vs baseline: 1.9298x; 1.0069x over previous
"""Trainium2 Bass kernel for nn_CrossAttention (B=4, Lq=Lk=2048, D=1024, H=16, d=64).

Sharding: 8 cores = 4 batches x 2 head-groups (8 heads each).
Each core computes a partial out^T = Wout_g^T @ y_g^T for its (batch, head-group);
host sums the two head-group partials per batch and transposes.

All matmuls run in bf16 (f32r costs ~2x bf16 on HW), the RoPE rotate-half is a
PE permutation matmul, and the projection phases are software-pipelined
(proj c | perm/var c-1 | bcast c-2) so the PE never waits on the
vector/scalar RMSNorm chain.  Both projection phases keep their input tiles
live simultaneously with all input DMAs issued up front — otherwise phase B's
loads alias phase A's SBUF and stall until A's last consumer retires (21us PE
gap + a HAM re-throttle).  PSUM rotates through one shared 8-bank pool.

Device layout is feature-major ("T" = [feature, seq]) throughout:
  qT/kT: [512, L] bf16 (8 heads x 64 dims on partitions, seq on free axis)
  S^T:   [k, q] tiles -> softmax sum via an appended ones-column in v (M=65)
  exp:   ACT, with the k-side RMSNorm rstd (and the 1/sqrt(d) scale) folded
         into the per-partition activation scale operand.
"""
import os
import numpy as np
from contextlib import ExitStack

import concourse.bass as bass
import concourse.tile as tile
from concourse import bacc, mybir
from concourse.bass_utils import run_bass_kernel_spmd

F32 = mybir.dt.float32
BF16 = mybir.dt.bfloat16
NP_BF16 = mybir.dt.np(BF16)
EXP = mybir.ActivationFunctionType.Exp
SQUARE = mybir.ActivationFunctionType.Square
SQRT = mybir.ActivationFunctionType.Sqrt
COPYF = mybir.ActivationFunctionType.Copy

D = 1024          # model dim
L = 2048          # seq len (q and k)
HC = 8            # heads per core
DH = 64           # head dim
F = HC * DH       # 512 local features
N_CORES = 8
EPS = float(np.finfo(np.float32).eps)

LAST_RESULTS = None  # BassKernelResults of the most recent run (for test harness)
_NC = None


# --------------------------------------------------------------------------- #
# Device program
# --------------------------------------------------------------------------- #

def _proj_dmas(nc, inp, x_dram, w_dram, c_dram, s_dram, side, wv_dram=None,
               tables=None):
    """Issue the input DMAs for one projection side; (w,x) interleaved per-dc
    so the first accumulation matmuls gate on the least data. `tables`
    shares another side's rope table tiles (pos_q == pos_k fast path)."""
    w_sb, x_sb = [], []
    for dc in range(8):
        w = inp.tile([128, F], BF16, tag=f"{side}w{dc}")
        nc.sync.dma_start(w[:], w_dram[dc * 128:(dc + 1) * 128, :])
        w_sb.append(w)
        x = inp.tile([128, L], BF16, tag=f"{side}x{dc}")
        nc.sync.dma_start(x[:], x_dram[dc * 128:(dc + 1) * 128, :])
        x_sb.append(x)
    wv_sb = []
    if wv_dram is not None:
        for dc in range(8):
            w = inp.tile([128, F], BF16, tag=f"{side}wv{dc}")
            nc.sync.dma_start(w[:], wv_dram[dc * 128:(dc + 1) * 128, :])
            wv_sb.append(w)
    if tables is not None:
        c_sb, s_sb = tables
    else:
        c_sb = inp.tile([128, L], BF16, tag=f"{side}ctab")
        nc.sync.dma_start(c_sb[:], c_dram[:])
        s_sb = inp.tile([128, L], BF16, tag=f"{side}stab")
        nc.sync.dma_start(s_sb[:], s_dram[:])
    return dict(w=w_sb, x=x_sb, wv=wv_sb, c=c_sb, s=s_sb)


def _proj_compute(tc, tiles, tmp, pps, dst, bdiag, bmap, perm, side,
                  rk_dram=None, vaug=None):
    """Project x (via w) into feature-major bf16 dst tiles [128, L] x4, with
    RMSNorm + RoPE applied. Software-pipelined over 16 chunks [128, 512].

    side == "q": multiply rstd into dst (via broadcast matmul).
    side == "k": write 0.125*rstd chunks to rk_dram instead (consumed by exp),
                 and also project v (tiles["wv"]) into vaug tiles.
    """
    nc = tc.nc
    w_sb, x_sb, wv_sb = tiles["w"], tiles["x"], tiles["wv"]
    c_sb, s_sb = tiles["c"], tiles["s"]
    eps_t = tmp.tile([2, 1], F32, tag=f"eps_{side}")
    nc.gpsimd.memset(eps_t[:], EPS if side == "q" else 64.0 * EPS)

    chunks = [(fb, qc) for fb in range(4) for qc in range(4)]
    st = [dict() for _ in chunks]   # per-chunk pipeline state

    def stage1(c):
        fb, qc = chunks[c]
        col0 = qc * 512
        ps = pps.tile([128, 512], F32, tag="proj", bufs=2)
        for dc in range(8):
            nc.tensor.matmul(ps[:],
                             w_sb[dc][:, fb * 128:(fb + 1) * 128],
                             x_sb[dc][:, col0:col0 + 512],
                             start=(dc == 0), stop=(dc == 7))
        raw = tmp.tile([128, 512], BF16, tag="raw", bufs=2)
        nc.vector.tensor_copy(raw[:], ps[:])          # cast for perm matmul
        sq = tmp.tile([128, 512], BF16, tag="sq", bufs=2)
        nc.scalar.activation(sq[:], ps[:], SQUARE)
        t1 = tmp.tile([128, 512], BF16, tag="t1", bufs=2)
        nc.gpsimd.tensor_mul(t1[:], raw[:], c_sb[:, col0:col0 + 512])
        st[c].update(ps=ps, raw=raw, sq=sq, t1=t1, col0=col0, fb=fb)

    def stage1v(kc):
        # v projection chunk kc -> vaug[kc] (seq-major), k side only
        ps = pps.tile([128, 512], F32, tag="aux", bufs=2)
        for dc in range(8):
            nc.tensor.matmul(ps[:],
                             x_sb[dc][:, kc * 128:(kc + 1) * 128],
                             wv_sb[dc][:],
                             start=(dc == 0), stop=(dc == 7))
        va = vaug[kc]
        nc.gpsimd.memset(va[:], 1.0)
        va3 = va.rearrange("p (h c) -> p h c", c=65)
        ps3 = ps.rearrange("p (h c) -> p h c", c=64)
        nc.vector.tensor_copy(va3[:, :, 0:64], ps3[:])

    def stage2(c):
        s = st[c]
        fb, col0 = s["fb"], s["col0"]
        rot = pps.tile([128, 512], F32, tag="rot", bufs=2)
        nc.tensor.matmul(rot[:], perm[:], s["raw"][:], start=True, stop=True)
        vps = pps.tile([2, 512], F32, tag="var", bufs=2)
        nc.tensor.matmul(vps[:], bdiag[:], s["sq"][:], start=True, stop=True)
        std = tmp.tile([2, 512], F32, tag="std", bufs=2)
        if side == "q":
            # std = sqrt(raw/64 + eps); rstd = 1/std
            nc.scalar.activation(std[:], vps[:], SQRT,
                                 bias=eps_t[:], scale=1.0 / 64.0)
        else:
            # fold the 1/8 attention scale: rk = 1/(8*std) = 1/sqrt(64*(raw/64+eps))
            nc.scalar.activation(std[:], vps[:], SQRT,
                                 bias=eps_t[:], scale=1.0)
        t2 = tmp.tile([128, 512], BF16, tag="t2", bufs=2)
        nc.vector.tensor_mul(t2[:], rot[:], s_sb[:, col0:col0 + 512])
        if side == "q":
            rstd = tmp.tile([2, 512], F32, tag="rstd", bufs=2)
            nc.vector.reciprocal_approx_fast(out=rstd[:], in_=std[:])
            rstd_b = tmp.tile([2, 512], BF16, tag="rstdb", bufs=2)
            nc.scalar.activation(rstd_b[:], rstd[:], COPYF)
            pre = tmp.tile([128, 512], BF16, tag="pre", bufs=2)
            nc.gpsimd.tensor_add(pre[:], s["t1"][:], t2[:])
            s.update(rstd_b=rstd_b, pre=pre)
        else:
            rstd = tmp.tile([2, 512], F32, tag="rstd", bufs=2)
            nc.vector.reciprocal_approx_fast(out=rstd[:], in_=std[:])
            # issue on the gpsimd queue: a data-dependent DMA on the Sync
            # queue would head-of-line-block phase B's input loads
            nc.gpsimd.dma_start(
                rk_dram[2 * fb:2 * fb + 2, col0:col0 + 512], rstd[:])
            nc.vector.tensor_add(dst[fb][:, col0:col0 + 512], s["t1"][:], t2[:])

    def stage3(c):
        # q only: broadcast rstd over the 2x64 head rows and multiply in
        s = st[c]
        fb, col0 = s["fb"], s["col0"]
        bps = pps.tile([128, 512], F32, tag="aux", bufs=2)
        nc.tensor.matmul(bps[:], bmap[:], s["rstd_b"][:], start=True, stop=True)
        nc.vector.tensor_mul(dst[fb][:, col0:col0 + 512], s["pre"][:], bps[:])
        st[c] = {}

    n = len(chunks)
    if side == "k":
        for i in range(n + 1):
            if i < n:
                stage1(i)
                stage1v(i)
            if i >= 1:
                stage2(i - 1)
    else:
        for i in range(n + 2):
            if i < n:
                stage1(i)
            if 1 <= i <= n:
                stage2(i - 1)
            if i >= 2:
                stage3(i - 2)


def _build_program(share_tables):
    nc = bacc.Bacc("TRN2", target_bir_lowering=False, debug=False,
                   num_devices=N_CORES)
    dt = nc.dram_tensor
    xqT = dt("xqT", (D, L), BF16, kind="ExternalInput").ap()
    xkvT = dt("xkvT", (D, L), BF16, kind="ExternalInput").ap()
    wq = dt("wq", (D, F), BF16, kind="ExternalInput").ap()
    wk = dt("wk", (D, F), BF16, kind="ExternalInput").ap()
    wv = dt("wv", (D, F), BF16, kind="ExternalInput").ap()
    wout = dt("wout", (F, D), BF16, kind="ExternalInput").ap()
    cq = dt("cq", (128, L), BF16, kind="ExternalInput").ap()
    sq_t = dt("sq", (128, L), BF16, kind="ExternalInput").ap()
    ck = dt("ck", (128, L), BF16, kind="ExternalInput").ap()
    sk_t = dt("sk", (128, L), BF16, kind="ExternalInput").ap()
    bdiag_d = dt("bdiag", (128, 2), BF16, kind="ExternalInput").ap()
    bmap_d = dt("bmap", (2, 128), BF16, kind="ExternalInput").ap()
    perm_d = dt("perm", (128, 128), BF16, kind="ExternalInput").ap()
    sel_d = [dt(f"sel{i}", (128, 128), BF16, kind="ExternalInput").ap()
             for i in range(2)]
    outT = dt("outT", (D, L), BF16, kind="ExternalOutput").ap()

    with tile.TileContext(nc) as tc:
        with ExitStack() as ctx:
            big = ctx.enter_context(tc.tile_pool(name="big", bufs=1))
            dram = ctx.enter_context(tc.tile_pool(name="dram", bufs=1, space="DRAM"))

            kT = [big.tile([128, L], BF16, tag=f"kT{i}", name=f"kT{i}") for i in range(4)]
            qT = [big.tile([128, L], BF16, tag=f"qT{i}", name=f"qT{i}") for i in range(4)]
            vaug = [big.tile([128, HC * 65], BF16, tag=f"v{i}", name=f"vaug{i}") for i in range(16)]
            rk_dram = dram.tile([HC, L], F32, tag="rk")
            # softmax denominators: per head-group tile, head h at partition
            # row 32*(h%4); 1/sums in bf16 for the phase-D broadcast
            sums_g = [big.tile([128, L], F32, tag=f"sums{g}", name=f"sums{g}")
                      for g in range(2)]
            nc.gpsimd.memset(sums_g[0][:], 1.0)
            nc.gpsimd.memset(sums_g[1][:], 1.0)
            rs_g = [big.tile([128, L], BF16, tag=f"rs{g}", name=f"rs{g}")
                    for g in range(2)]
            rk_sb = big.tile([128, HC, 16], F32, tag="rk_sb")

            bdiag = big.tile([128, 2], BF16, tag="bdiag")
            nc.sync.dma_start(bdiag[:], bdiag_d[:])
            bmap = big.tile([2, 128], BF16, tag="bmap")
            nc.sync.dma_start(bmap[:], bmap_d[:])
            perm = big.tile([128, 128], BF16, tag="perm")
            nc.sync.dma_start(perm[:], perm_d[:])

            # ---- Phases A+B: projections ----
            with ExitStack() as pctx:
                inp = pctx.enter_context(tc.tile_pool(name="inp", bufs=1))
                tmp = pctx.enter_context(tc.tile_pool(name="tmp", bufs=1))
                pps = pctx.enter_context(
                    tc.tile_pool(name="proj_ps", bufs=1, space="PSUM"))
                a_tiles = _proj_dmas(nc, inp, xkvT, wk, ck, sk_t, "k",
                                     wv_dram=wv)
                b_tiles = _proj_dmas(
                    nc, inp, xqT, wq, cq, sq_t, "q",
                    tables=((a_tiles["c"], a_tiles["s"])
                            if share_tables else None))
                _proj_compute(tc, a_tiles, tmp, pps, kT, bdiag, bmap, perm,
                              side="k", rk_dram=rk_dram, vaug=vaug)
                _proj_compute(tc, b_tiles, tmp, pps, qT, bdiag, bmap, perm,
                              side="q")

            # rk transpose gather (Sync queue, after all input loads)
            nc.sync.dma_start(
                rk_sb[:], rk_dram.rearrange("h (kc p) -> p h kc", p=128))

            ytr = [big.tile([128, L], BF16, tag=f"ytr{i}", name=f"ytr{i}")
                   for i in range(4)]

            # ---- Phases C+D ----
            with ExitStack() as cctx:
                cpool = cctx.enter_context(tc.tile_pool(name="cd_sb", bufs=1))
                cps = cctx.enter_context(
                    tc.tile_pool(name="att_ps", bufs=1, space="PSUM"))
                wo_sb = []
                for fc in range(4):
                    w = cpool.tile([128, D], BF16, tag=f"wo{fc}")
                    nc.sync.dma_start(w[:], wout[fc * 128:(fc + 1) * 128, :])
                    wo_sb.append(w)
                sel_sb = []
                for i in range(2):
                    s = cpool.tile([128, 128], BF16, tag=f"sel{i}")
                    nc.sync.dma_start(s[:], sel_d[i][:])
                    sel_sb.append(s)

                # ---- Phase C: attention ----
                va3s = [vaug[kc].rearrange("p (h c) -> p h c", c=65)
                        for kc in range(16)]
                for h in range(HC):
                    fb, off = h // 2, (h % 2) * 64
                    yps = [cps.tile([128, 512], F32, tag=f"y{qn}", bufs=1,
                                    name=f"yps{h}_{qn}")[0:65, :]
                           for qn in range(4)]
                    pend = None   # (kc, [pt_half0, pt_half1]) awaiting attnv
                    for kc in range(16):
                        pts = []
                        for half in range(2):
                            sps = cps.tile([128, 1024], F32, tag="s", bufs=2)
                            for j in range(2):
                                qn = half * 2 + j
                                nc.tensor.matmul(
                                    sps[:, j * 512:(j + 1) * 512],
                                    kT[fb][off:off + 64,
                                           kc * 128:(kc + 1) * 128],
                                    qT[fb][off:off + 64,
                                           qn * 512:(qn + 1) * 512],
                                    start=True, stop=True)
                            pt = cpool.tile([128, 1024], BF16, tag="p", bufs=4)
                            nc.scalar.activation(pt[:], sps[:], EXP,
                                                 scale=rk_sb[:, h, kc:kc + 1])
                            pts.append(pt)
                        if pend is not None:
                            pkc, ppts = pend
                            for half in range(2):
                                for j in range(2):
                                    qn = half * 2 + j
                                    nc.tensor.matmul(
                                        yps[qn][:], va3s[pkc][:, h, :],
                                        ppts[half][:, j * 512:(j + 1) * 512],
                                        start=(pkc == 0), stop=False)
                        pend = (kc, pts)
                    pkc, ppts = pend
                    for half in range(2):
                        for j in range(2):
                            qn = half * 2 + j
                            nc.tensor.matmul(
                                yps[qn][:], va3s[pkc][:, h, :],
                                ppts[half][:, j * 512:(j + 1) * 512],
                                start=False, stop=True)
                    slot = 32 * (h % 4)
                    g = h // 4
                    for qn in range(4):
                        sl = slice(qn * 512, (qn + 1) * 512)
                        nc.vector.tensor_copy(ytr[fb][off:off + 64, sl],
                                              yps[qn][0:64, :])
                        nc.vector.tensor_copy(sums_g[g][slot:slot + 1, sl],
                                              yps[qn][64:65, :])
                        if h in (3, 7):
                            # group complete: fold 1/sums per qn chunk while
                            # attention (or phase D stage 1) is still pending
                            rs32 = cpool.tile([128, 512], F32, tag="rs32",
                                              bufs=2)
                            nc.vector.reciprocal_approx_fast(
                                out=rs32[:], in_=sums_g[g][:, sl])
                            nc.vector.tensor_copy(rs_g[g][:, sl], rs32[:])

                # ---- Phase D: normalize + output projection (per-qn) ----
                def d_stage1(qn):
                    sl = slice(qn * 512, (qn + 1) * 512)
                    for pair in range(2):
                        bt = cps.tile([128, 1024], F32, tag="s", bufs=2,
                                      name=f"bc2_{pair}_{qn}")
                        for half in range(2):
                            fb = pair * 2 + half
                            bps = bt[:, half * 512:(half + 1) * 512]
                            nc.tensor.matmul(bps, sel_sb[fb % 2][:],
                                             rs_g[fb // 2][:, sl],
                                             start=True, stop=True)
                            nc.vector.tensor_mul(ytr[fb][:, sl],
                                                 ytr[fb][:, sl], bps)

                def d_stage2(qn):
                    sl = slice(qn * 512, (qn + 1) * 512)
                    for nb in range(8):
                        ps = cps.tile([128, 512], F32, tag=f"y{nb % 4}",
                                      bufs=1, name=f"oproj_{nb}_{qn}")
                        for fc in range(4):
                            nc.tensor.matmul(
                                ps[:],
                                wo_sb[fc][:, nb * 128:(nb + 1) * 128],
                                ytr[fc][:, sl],
                                start=(fc == 0), stop=(fc == 3))
                        ot = cpool.tile([128, 512], BF16, tag="ot", bufs=4)
                        nc.vector.tensor_copy(ot[:], ps[:])
                        eng = nc.sync if nb % 2 == 0 else nc.gpsimd
                        eng.dma_start(
                            outT[nb * 128:(nb + 1) * 128, sl], ot[:])

                for i in range(5):
                    if i < 4:
                        d_stage1(i)
                    if i >= 1:
                        d_stage2(i - 1)
    nc.compile()
    return nc


def get_nc(share_tables=True):
    global _NC
    if _NC is None or _NC[1] != share_tables:
        _NC = (_build_program(share_tables), share_tables)
    return _NC[0]


# --------------------------------------------------------------------------- #
# Host side
# --------------------------------------------------------------------------- #

def _rope_tables(pos, g):
    """Feature-major folded RoPE(+gain) tables, replicated for a 2-head tile."""
    pos = np.asarray(pos).astype(np.float32)
    g = np.asarray(g, dtype=np.float32)
    inv = (1.0 / (10000.0 ** (np.arange(0, DH, 2, dtype=np.float32)
                              / np.float32(DH)))).astype(np.float32)
    ang = pos[:, None] * inv[None, :]                      # (L, 32)
    cos, sin = np.cos(ang, dtype=np.float32), np.sin(ang, dtype=np.float32)
    j = np.arange(DH)
    C = (g[j][:, None] * cos[:, j % 32].T).astype(np.float32)       # (64, L)
    sign = np.where(j < 32, -1.0, 1.0).astype(np.float32)
    S = (sign[:, None] * g[(j + 32) % 64][:, None]
         * sin[:, j % 32].T).astype(np.float32)
    return (np.ascontiguousarray(np.tile(C, (2, 1))).astype(NP_BF16),
            np.ascontiguousarray(np.tile(S, (2, 1))).astype(NP_BF16))


def make_in_maps(queries, kv, Wq, Wkv, Wout, g_q, g_k, pos_q, pos_k):
    queries = np.asarray(queries, dtype=np.float32)
    kv = np.asarray(kv, dtype=np.float32)
    Wq = np.asarray(Wq, dtype=np.float32)
    Wkv = np.asarray(Wkv, dtype=np.float32)
    Wout = np.asarray(Wout, dtype=np.float32)

    cq, sq = _rope_tables(pos_q, g_q)
    ck, sk = _rope_tables(pos_k, g_k)
    bdiag = np.zeros((128, 2), np.float32)
    bdiag[0:64, 0] = 1.0
    bdiag[64:128, 1] = 1.0
    bmap = np.zeros((2, 128), np.float32)
    bmap[0, 0:64] = 1.0
    bmap[1, 64:128] = 1.0
    # unsigned rotate-half permutation (sign lives in the S table):
    # rot[i] = raw[i+32] for i%64<32 else raw[i-32]
    perm = np.zeros((128, 128), np.float32)
    for i in range(128):
        src = i + 32 if (i % 64) < 32 else i - 32
        perm[src, i] = 1.0
    # sums-row selectors: within its group tile, head h's denominators live
    # at row 32*(h%4); ytr[fb] rows 0:64 = head 2fb, 64:128 = head 2fb+1
    selA = np.zeros((128, 128), np.float32)
    selA[0, 0:64] = 1.0
    selA[32, 64:128] = 1.0
    selB = np.zeros((128, 128), np.float32)
    selB[64, 0:64] = 1.0
    selB[96, 64:128] = 1.0

    Wkv3 = Wkv.reshape(D, 16, 2 * DH)
    in_maps = []
    for c in range(N_CORES):
        b, grp = c // 2, c % 2
        hs = slice(grp * HC, (grp + 1) * HC)
        im = {
            "xqT": np.ascontiguousarray(queries[b].T).astype(NP_BF16),
            "xkvT": np.ascontiguousarray(kv[b].T).astype(NP_BF16),
            "wq": np.ascontiguousarray(
                Wq[:, grp * F:(grp + 1) * F]).astype(NP_BF16),
            "wk": np.ascontiguousarray(
                Wkv3[:, hs, :DH].reshape(D, F)).astype(NP_BF16),
            "wv": np.ascontiguousarray(
                Wkv3[:, hs, DH:].reshape(D, F)).astype(NP_BF16),
            "wout": np.ascontiguousarray(
                Wout[grp * F:(grp + 1) * F, :]).astype(NP_BF16),
            "cq": cq, "sq": sq, "ck": ck, "sk": sk,
            "bdiag": bdiag.astype(NP_BF16), "bmap": bmap.astype(NP_BF16),
            "perm": perm.astype(NP_BF16),
            "sel0": selA.astype(NP_BF16), "sel1": selB.astype(NP_BF16),
        }
        in_maps.append(im)
    return in_maps


def kernel(queries, kv, Wq, Wkv, Wout, g_q, g_k, pos_q, pos_k):
    global LAST_RESULTS
    share = bool(np.array_equal(np.asarray(pos_q), np.asarray(pos_k))
                 and np.array_equal(np.asarray(g_q), np.asarray(g_k)))
    nc = get_nc(share)
    in_maps = make_in_maps(queries, kv, Wq, Wkv, Wout, g_q, g_k, pos_q, pos_k)
    trace = bool(int(os.environ.get("KERNEL_TRACE", "0")))
    kw = {}
    if trace:
        kw["tmpdir"] = os.environ.get("KERNEL_TRACE_DIR") or None
    res = run_bass_kernel_spmd(nc, in_maps, core_ids=list(range(N_CORES)),
                               trace=trace, **kw)
    LAST_RESULTS = res
    out = np.empty((4, L, D), np.float32)
    for b in range(4):
        out[b] = (res.results[2 * b]["outT"].astype(np.float32)
                  + res.results[2 * b + 1]["outT"].astype(np.float32)).T
    return out
